# revision 18
# baseline (speedup 1.0000x reference)
"""BiViewMixHop GNN kernel for 8 Trainium2 NeuronCores (Bass/Tile).

Strategy (v2):
  - Algebraic restructure: P(h)@W1 + P^2(h)@W2 = P(h@W1 + P(h@W2)); hom/het
    views fused into one 128-col tensor -> 2 gather passes per layer (6 total).
  - Host prep (index manipulation only): relabel nodes into graph-aligned
    32-slot-padded "slots", shard whole graphs contiguously across 8 cores,
    sort each core's edges by dst slot, pad each 32-slot group's edge list to
    a multiple of 128 ("chunks"). Chunk counts per group are maxed across
    cores so ONE SPMD program serves all 8 cores.
  - Gather: batched SWDGE dma_gather (mlp ucode) -- ONE instruction per
    ~64 chunks (8192 edges) instead of one indirect DMA per chunk. Tables
    (c/u) are bf16, PAIR-PACKED: one 512B row = two nodes' 128 bf16 feats,
    so int16 indices (= slot//2) cover the 54k-slot space.
  - Scatter/segment-sum: per chunk, a single one-hot "eq" matrix (dstloc ==
    iota) is the PE stationary operand; the per-edge hom/het masks (split by
    slot parity to pick the right half of each gathered pair) are multiplied
    into the gathered tile on DVE (broadcast tensor_tensor, one op per
    gather batch). Two N=128 matmuls per chunk accumulate even+odd message
    sums into one PSUM tile.
  - Degree normalization: 1/deg precomputed on host, applied as per-partition
    ACT scale at PSUM evacuation.
  - Cross-core exchange of gather-source tensors via bf16 DRAM AllGather.
  - Readout: mean-pool via PE matmuls against one-hot graph selectors;
    max-pool via windowed reduce + per-graph mask+reduce; combined across
    cores with AllReduce; small MLP head + log_softmax replicated per core.
"""

import numpy as np

# ---------------------------------------------------------------- constants
F_IN = 128
H_HID = 64
N_LAYERS = 3
N_GRAPHS = 256
N_CLS = 10
NCORES = 8
P = 128
DGRP = 32           # dst-group width (eq-matrix columns)
SBATCH = 16         # chunks per eq-build batch
BCH = 8             # chunks per dma_gather batch (1024 idxs: SWDGE sweet spot)
W_G = 64            # per-core local-graph window
NQ = 4              # SWDGE queues
NGP = N_GRAPHS + 8  # pooled-buffer rows (graphs + dump row for pads)
SKIP_GATHER = False  # timing knockout: memset instead of dma_gather
SKIP_AG = False      # timing knockout: skip AllGathers


# ------------------------------------------------------------------ host prep
def _prep(edge_index, batch, hom_mask, het_mask):
    """Index-only preprocessing. Returns structure dict + per-core arrays."""
    import ml_dtypes
    bf16 = ml_dtypes.bfloat16

    N = batch.shape[0]
    src, dst = np.asarray(edge_index[0]), np.asarray(edge_index[1])
    batch = np.asarray(batch)
    counts = np.bincount(batch, minlength=N_GRAPHS)

    pad_sizes = ((counts + DGRP - 1) // DGRP) * DGRP
    total_slots = int(pad_sizes.sum())
    target = max(total_slots / NCORES, 1.0)

    csum = np.concatenate([[0], np.cumsum(pad_sizes)])
    gcore = np.minimum((csum[:-1] + pad_sizes / 2) / target, NCORES - 1).astype(np.int64)
    gcore = np.maximum.accumulate(gcore)

    core_slots = np.zeros(NCORES, np.int64)
    graph_base = np.zeros(N_GRAPHS, np.int64)
    for g in range(N_GRAPHS):
        c = gcore[g]
        graph_base[g] = core_slots[c]
        core_slots[c] += pad_sizes[g]
    S_core = int(((core_slots.max() + P - 1) // P) * P)
    TILES = S_core // P
    GROUPS = S_core // DGRP
    TOT = NCORES * S_core
    assert TOT % 2 == 0 and TOT // 2 < 32768, "pair-packed idx must fit int16"

    node_starts = np.concatenate([[0], np.cumsum(counts)])
    node_slot = np.empty(N, np.int64)
    for g in range(N_GRAPHS):
        a, b = node_starts[g], node_starts[g + 1]
        if b > a:
            base = gcore[g] * S_core + graph_base[g]
            node_slot[a:b] = base + np.arange(b - a)

    src_slot = node_slot[src]
    dst_slot = node_slot[dst]
    owner = dst_slot // S_core

    core_graphs = [np.where((gcore == c) & (counts > 0))[0] for c in range(NCORES)]
    n_loc = max(len(cg) for cg in core_graphs)
    assert n_loc <= W_G, f"{n_loc} local graphs > W_G={W_G}"

    cpg = np.ones(GROUPS, np.int64)
    per_core_edges = []
    hom_mask = np.asarray(hom_mask)
    het_mask = np.asarray(het_mask)
    for c in range(NCORES):
        m = owner == c
        es, ed = src_slot[m], dst_slot[m]
        eh, et = hom_mask[m], het_mask[m]
        order = np.argsort(ed, kind="stable")
        es, ed, eh, et = es[order], ed[order], eh[order], et[order]
        loc = ed - c * S_core
        grp = loc // DGRP
        gstart = np.searchsorted(grp, np.arange(GROUPS))
        gend = np.searchsorted(grp, np.arange(GROUPS) + 1)
        per_core_edges.append((es, loc, eh, et, gstart, gend))
        cnt = gend - gstart
        cpg = np.maximum(cpg, (cnt + P - 1) // P)
    C_CH = int(cpg.sum())
    cog = np.concatenate([[0], np.cumsum(cpg)])
    NIDX = C_CH * P
    IDXCOLS = NIDX // 16

    st = {"S_core": S_core, "TILES": TILES, "GROUPS": GROUPS, "TOT": TOT,
          "C_CH": C_CH, "cpg": cpg, "cog": cog, "node_slot": node_slot,
          "counts": counts, "n_loc": n_loc, "IDXCOLS": IDXCOLS}

    per_core = []
    for c in range(NCORES):
        es, loc, eh, et, gstart, gend = per_core_edges[c]
        srcA = np.zeros((P, C_CH), np.int64)
        dlA = np.zeros((P, C_CH), np.float32)
        mhA = np.zeros((P, C_CH), np.float32)
        mtA = np.zeros((P, C_CH), np.float32)
        for g in range(GROUPS):
            a, b = gstart[g], gend[g]
            c0 = cog[g]
            for j in range(cpg[g]):
                lo = a + j * P
                hi = min(a + (j + 1) * P, b)
                if hi <= lo:
                    break
                k = hi - lo
                srcA[:k, c0 + j] = es[lo:hi]
                dlA[:k, c0 + j] = (loc[lo:hi] - g * DGRP).astype(np.float32)
                mhA[:k, c0 + j] = eh[lo:hi]
                mtA[:k, c0 + j] = et[lo:hi]

        # int16 pair indices, wrapped into 16 partitions column-major and
        # replicated across the 8 Q7 cores' partition groups
        flat = (srcA // 2).astype(np.int16).flatten(order="F")  # pos i=(p,j)
        idx16 = np.zeros((P, IDXCOLS), np.int16)
        base16 = flat.reshape(IDXCOLS, 16).T
        for k in range(8):
            idx16[16 * k:16 * (k + 1), :] = base16

        # per-edge masks split by src-slot parity, interleaved per chunk as
        # (even-hom, even-het, odd-hom, odd-het) quads
        par = (srcA % 2).astype(np.float32)
        mq = np.zeros((P, 4 * C_CH), np.float32)
        mq[:, 0::4] = mhA * (1.0 - par)
        mq[:, 1::4] = mtA * (1.0 - par)
        mq[:, 2::4] = mhA * par
        mq[:, 3::4] = mtA * par

        # 1/deg per dst slot, hom/het, laid out [128, 2*TILES]
        dst_all = loc
        rdeg = np.zeros((P, 2 * TILES), np.float32)
        dh = 1.0 / np.maximum(np.bincount(dst_all, weights=eh, minlength=S_core), 1.0)
        dt = 1.0 / np.maximum(np.bincount(dst_all, weights=et, minlength=S_core), 1.0)
        rdeg[:, 0::2] = dh.reshape(TILES, P).T
        rdeg[:, 1::2] = dt.reshape(TILES, P).T

        batchloc = np.full(S_core, -1.0, np.float32)
        maskrow = np.zeros((W_G, GROUPS), np.float32)
        gidx = np.full((W_G, 1), N_GRAPHS, np.int32)  # pads -> dump row
        for li, g in enumerate(core_graphs[c]):
            base = graph_base[g]
            batchloc[base:base + counts[g]] = li
            g0, g1 = base // DGRP, (base + pad_sizes[g]) // DGRP
            maskrow[li, g0:g1] = 1.0
            gidx[li, 0] = g
        per_core.append({
            "idx16": idx16,
            "dl": dlA.astype(bf16),
            "mq": mq.astype(bf16),
            "rdeg": rdeg,
            "batchloc": batchloc.reshape(TILES, P).T.copy(),
            "maskrow": maskrow.reshape(1, -1).copy(), "gidx": gidx})
    return st, per_core


def _host_consts(st):
    """Constant helper tensors (iota patterns, identity)."""
    import ml_dtypes
    bf16 = ml_dtypes.bfloat16
    iotaDG = np.tile(np.arange(DGRP, dtype=np.float32), SBATCH)[None, :].repeat(P, 0)
    iotaWG = np.arange(W_G, dtype=np.float32)[None, :].repeat(P, 0)
    ident = np.eye(P, dtype=np.float32)
    return {"iotaDG": iotaDG.astype(bf16), "iotaWG": iotaWG, "identA": ident}


# ------------------------------------------------------------- device builder
def _build(st):
    import concourse.bass as bass
    import concourse.bacc as bacc
    import concourse.mybir as mybir
    import concourse.tile as tile

    S_core, TILES, GROUPS = st["S_core"], st["TILES"], st["GROUPS"]
    TOT, C_CH, cog = st["TOT"], st["C_CH"], st["cog"]
    IDXCOLS, N_LOC = st["IDXCOLS"], st["n_loc"]
    NB = (C_CH + BCH - 1) // BCH      # gather batches per pass
    NSB = (C_CH + SBATCH - 1) // SBATCH
    f32 = mybir.dt.float32
    bf16 = mybir.dt.bfloat16
    F2 = 2 * H_HID  # 128

    nc = bacc.Bacc("TRN2", target_bir_lowering=False, debug=False,
                   num_devices=NCORES, num_swdge_queues=NQ)

    def din(name, shape, dtype=f32):
        return nc.dram_tensor(name, shape, dtype, kind="ExternalInput").ap()

    x_own = din("x_own", [S_core, F_IN])
    idx16 = din("idx16", [P, IDXCOLS], mybir.dt.int16)
    dlA = din("dlA", [P, C_CH], bf16)
    mqA = din("mqA", [P, 4 * C_CH], bf16)
    rdegA = din("rdegA", [P, 2 * TILES])
    batchloc = din("batchloc", [P, TILES])
    maskrow = din("maskrow", [1, W_G * GROUPS])
    gidx = din("gidx", [W_G, 1], mybir.dt.int32)
    rcount_g = din("rcount_g", [NGP, 1])
    iotaDG_d = din("iotaDG", [P, SBATCH * DGRP], bf16)
    iotaWG_d = din("iotaWG", [P, W_G])
    identA = din("identA", [P, P])
    w2cat = din("w2cat", [N_LAYERS, F_IN, F2])
    w1cat = din("w1cat", [N_LAYERS, F_IN, F2])
    w0cat = din("w0cat", [N_LAYERS, F_IN, F2])
    bcat = din("bcat", [N_LAYERS, 1, F2])
    lin1w = din("lin1w", [4 * H_HID, 2 * H_HID])
    lin1b = din("lin1b", [2 * H_HID, 1])
    lin2w = din("lin2w", [2 * H_HID, H_HID])
    lin2b = din("lin2b", [H_HID, 1])
    lin3w = din("lin3w", [H_HID, N_CLS])
    lin3b = din("lin3b", [N_CLS, 1])

    out = nc.dram_tensor("out", [N_GRAPHS, N_CLS], f32, kind="ExternalOutput").ap()

    c_own = nc.dram_tensor("c_own", [S_core, F_IN], bf16).ap()
    u_own = nc.dram_tensor("u_own", [S_core, F_IN], bf16).ap()
    # pair-packed gather tables: one row = two consecutive slots' features
    c_full = nc.dram_tensor("c_full", [TOT // 2, 2 * F_IN], bf16,
                            addr_space="Shared").ap()
    u_full = nc.dram_tensor("u_full", [TOT // 2, 2 * F_IN], bf16,
                            addr_space="Shared").ap()
    maxbuf = nc.dram_tensor("maxbuf", [NGP, F_IN], f32).ap()
    sumbuf = nc.dram_tensor("sumbuf", [NGP, F_IN], f32).ap()
    maxbuf_o = nc.dram_tensor("maxbuf_o", [NGP, F_IN], f32, addr_space="Shared").ap()
    sumbuf_o = nc.dram_tensor("sumbuf_o", [NGP, F_IN], f32, addr_space="Shared").ap()

    with tile.TileContext(nc) as tc:
        with tc.tile_pool(name="const", bufs=1) as cpool, \
             tc.tile_pool(name="sb", bufs=3) as spool, \
             tc.tile_pool(name="gt", bufs=8) as gpool, \
             tc.tile_pool(name="stg", bufs=3) as stgpool, \
             tc.tile_pool(name="sm", bufs=3) as smpool, \
             tc.tile_pool(name="psA", bufs=3, space="PSUM") as psA, \
             tc.tile_pool(name="psB", bufs=2, space="PSUM") as psB, \
             tc.tile_pool(name="psC", bufs=2, space="PSUM") as psC, \
             tc.tile_pool(name="psP", bufs=1, space="PSUM") as psP:

            # ---------------- resident tiles
            ident = cpool.tile([P, P], f32)
            nc.sync.dma_start(ident[:], identA[:])
            idx_t = cpool.tile([P, IDXCOLS], mybir.dt.int16)
            nc.sync.dma_start(idx_t[:], idx16[:])
            dl_t = cpool.tile([P, C_CH], bf16)
            nc.sync.dma_start(dl_t[:], dlA[:])
            mq_t = cpool.tile([P, 4 * C_CH], bf16)
            nc.sync.dma_start(mq_t[:], mqA[:])
            iota32 = cpool.tile([P, SBATCH * DGRP], bf16)
            nc.sync.dma_start(iota32[:], iotaDG_d[:])
            iotaWG = cpool.tile([P, W_G], f32)
            nc.sync.dma_start(iotaWG[:], iotaWG_d[:])
            ones_col = cpool.tile([1, P], f32)
            nc.vector.memset(ones_col[:], 1.0)
            zero_t = cpool.tile([P, P], f32)
            nc.vector.memset(zero_t[:], 0.0)
            hT = cpool.tile([P, S_core], f32)
            rdeg = cpool.tile([P, 2 * TILES], f32)
            nc.sync.dma_start(rdeg[:], rdegA[:])
            bl_t = cpool.tile([P, TILES], f32)
            nc.sync.dma_start(bl_t[:], batchloc[:])
            mrowP = cpool.tile([1, N_LOC * GROUPS], f32)
            nc.sync.dma_start(mrowP[:], maskrow[:, 0:N_LOC * GROUPS])
            gidx_t = cpool.tile([W_G, 1], mybir.dt.int32)
            nc.sync.dma_start(gidx_t[:], gidx[:])
            rcg_t = cpool.tile([P, 2], f32)
            nc.sync.dma_start(
                rcg_t[:], rcount_g[:2 * P, :].rearrange("(a b) o -> b (a o)", a=2))
            wAll = cpool.tile([P, 9 * F2], f32)
            for l in range(N_LAYERS):
                nc.sync.dma_start(wAll[:, (3 * l + 0) * F2:(3 * l + 1) * F2], w2cat[l])
                nc.sync.dma_start(wAll[:, (3 * l + 1) * F2:(3 * l + 2) * F2], w1cat[l])
                nc.sync.dma_start(wAll[:, (3 * l + 2) * F2:(3 * l + 3) * F2], w0cat[l])
            bAll = cpool.tile([1, N_LAYERS * F2], f32)
            for l in range(N_LAYERS):
                nc.sync.dma_start(bAll[:, l * F2:(l + 1) * F2], bcat[l])
            l1w = cpool.tile([P, 2 * F2], f32)  # two K-halves side by side
            nc.sync.dma_start(l1w[:, 0:F2], lin1w[0:P, :])
            nc.sync.dma_start(l1w[:, F2:2 * F2], lin1w[P:2 * P, :])
            l2w = cpool.tile([2 * H_HID, H_HID], f32)
            nc.sync.dma_start(l2w[:], lin2w[:])
            l2b = cpool.tile([H_HID, 1], f32)
            nc.sync.dma_start(l2b[:], lin2b[:])
            l1b = cpool.tile([2 * H_HID, 1], f32)
            nc.sync.dma_start(l1b[:], lin1b[:])
            l3w = cpool.tile([H_HID, N_CLS], f32)
            nc.sync.dma_start(l3w[:], lin3w[:])
            l3b = cpool.tile([N_CLS, 1], f32)
            nc.sync.dma_start(l3b[:], lin3b[:])
            spool_t = cpool.tile([P, TILES * W_G], f32)
            gmax12 = cpool.tile([P, W_G], f32)
            nc.vector.memset(gmax12[:], 0.0)

            def wslice(l, which):  # 0=w2, 1=w1, 2=w0
                o = (3 * l + which) * F2
                return wAll[:, o:o + F2]

            # ---------------- x -> hT (feature-major)
            for t in range(TILES):
                xs = stgpool.tile([P, P], f32, tag="xs")
                nc.sync.dma_start(xs[:], x_own[t * P:(t + 1) * P, :])
                pt = psC.tile([P, 2 * P], f32, tag="c")
                nc.tensor.transpose(pt[:, 0:P], xs[:], ident[:])
                nc.scalar.copy(hT[:, t * P:(t + 1) * P], pt[:, 0:P])

            # ---------------- S_pool (one-hot local-graph selectors)
            for t in range(TILES):
                nc.vector.tensor_tensor(
                    out=spool_t[:, t * W_G:(t + 1) * W_G],
                    in0=bl_t[:, t:t + 1].to_broadcast([P, W_G]),
                    in1=iotaWG[:], op=mybir.AluOpType.is_equal)

            # ---------------- zero-fill pooled buffers
            for buf in (maxbuf, sumbuf):
                r = 0
                while r < NGP:
                    k = min(P, NGP - r)
                    nc.sync.dma_start(buf[r:r + k, :], zero_t[:k, :])
                    r += k

            pool_ps = psP.tile([W_G, F_IN], f32)

            def produce(l, dest):
                for t in range(TILES):
                    pc = psC.tile([P, 2 * P], f32, tag="c")
                    nc.tensor.matmul(pc[:, 0:F2], lhsT=hT[:, t * P:(t + 1) * P],
                                     rhs=wslice(l, 0), start=True, stop=True)
                    cs = stgpool.tile([P, F2], bf16, tag="cs")
                    nc.scalar.copy(cs[:], pc[:, 0:F2])
                    nc.sync.dma_start(dest[t * P:(t + 1) * P, :], cs[:])

            def build_eq(b):
                c0 = b * SBATCH
                nch = min(SBATCH, C_CH - c0)
                eq = spool.tile([P, SBATCH * DGRP], bf16, tag="eq")
                r3 = lambda ap: ap.rearrange("p (k d) -> p k d", d=DGRP)[:, :nch, :]
                nc.vector.tensor_tensor(
                    out=r3(eq[:]),
                    in0=dl_t[:, c0:c0 + nch, None].to_broadcast([P, nch, DGRP]),
                    in1=r3(iota32[:]), op=mybir.AluOpType.is_equal)
                return eq

            nreg_cache = {}

            def nreg(n):
                if n not in nreg_cache:
                    nreg_cache[n] = nc.gpsimd.to_reg(n)
                return nreg_cache[n]

            def prop_pass(l, table, mode, do_pool):
                # ---- issue all gather batches (pool bufs throttle pipeline)
                gts = []
                for b in range(NB):
                    j0 = b * BCH
                    nch = min(BCH, C_CH - j0)
                    gt = gpool.tile([P, BCH * 2 * F_IN], bf16, tag="g")
                    out3 = gt[:, :nch * 2 * F_IN].rearrange(
                        "p (c e) -> p c e", e=2 * F_IN)
                    if SKIP_GATHER:
                        nc.vector.memset(gt[:, :nch * 2 * F_IN], 0.25)
                    else:
                        nc.gpsimd.dma_gather(
                            out3, table, idx_t[:, j0 * 8:(j0 + nch) * 8],
                            nch * P, nreg(nch * P), 2 * F_IN, queue_num=b % NQ,
                            single_packet=False)
                    gts.append(gt)

                def mask_mult(b):
                    # fold per-edge masks (parity x view) into gathered tile;
                    # emitted lazily in consume order so the DVE stream stays
                    # deadlock-free w.r.t. the eq builds.
                    j0 = b * BCH
                    nch = min(BCH, C_CH - j0)
                    gt = gts[b]
                    m4 = gt[:, :nch * 2 * F_IN].rearrange(
                        "p (c s) -> p c s", s=H_HID)
                    nc.vector.tensor_tensor(
                        out=m4,
                        in0=m4,
                        in1=mq_t[:, 4 * j0:4 * (j0 + nch), None].to_broadcast(
                            [P, 4 * nch, H_HID]),
                        op=mybir.AluOpType.mult)

                eq_cache = {}

                def get_eq(b):
                    if b not in eq_cache:
                        # build one sbatch ahead so PE never waits on a
                        # freshly-emitted DVE op
                        for k in (b, min(b + 1, NSB - 1)):
                            if k not in eq_cache:
                                eq_cache[k] = build_eq(k)
                        for k in [k for k in eq_cache if k < b - 1]:
                            del eq_cache[k]
                    return eq_cache[b]

                masked_upto = [-1]
                LOOKAHEAD = 4  # < gpool bufs to stay deadlock-free

                def ensure_masked(b):
                    tgt = min(b + LOOKAHEAD, NB - 1)
                    while masked_upto[0] < tgt:
                        masked_upto[0] += 1
                        mask_mult(masked_upto[0])

                for t in range(TILES):
                    stg = stgpool.tile([P, P], f32, tag="hstg")
                    for gi in range(4):
                        g = t * 4 + gi
                        ps_t = psA.tile([DGRP, F_IN], f32, tag="t")
                        chunks = list(range(cog[g], cog[g + 1]))
                        for j in chunks:
                            eq = get_eq(j // SBATCH)
                            jj = j % SBATCH
                            ensure_masked(j // BCH)
                            gt = gts[j // BCH]
                            jb = j % BCH
                            lhs = eq[:, jj * DGRP:(jj + 1) * DGRP]
                            nc.tensor.matmul(
                                ps_t[:],
                                lhsT=lhs,
                                rhs=gt[:, jb * 2 * F_IN:jb * 2 * F_IN + F_IN],
                                start=j == chunks[0], stop=False)
                            nc.tensor.matmul(
                                ps_t[:],
                                lhsT=lhs,
                                rhs=gt[:, jb * 2 * F_IN + F_IN:(jb + 1) * 2 * F_IN],
                                start=False, stop=j == chunks[-1])
                        ps_a = psB.tile([DGRP, F_IN], f32, tag="a")
                        nc.tensor.matmul(
                            ps_a[:], lhsT=hT[:, g * DGRP:(g + 1) * DGRP],
                            rhs=wslice(l, 1 if mode == "A" else 2),
                            start=True, stop=(mode == "A"))
                        if mode == "B":
                            nc.tensor.matmul(ps_a[:], lhsT=ones_col[:, 0:DGRP],
                                             rhs=bAll[:, l * F2:(l + 1) * F2],
                                             start=False, stop=True)
                        r0 = (g % 4) * DGRP
                        t0 = g // 4
                        tp = smpool.tile([DGRP, F_IN], f32, tag="tp")
                        nc.scalar.mul(tp[:, 0:H_HID], ps_t[:, 0:H_HID],
                                      rdeg[r0:r0 + DGRP, 2 * t0:2 * t0 + 1])
                        nc.scalar.mul(tp[:, H_HID:F_IN], ps_t[:, H_HID:F_IN],
                                      rdeg[r0:r0 + DGRP, 2 * t0 + 1:2 * t0 + 2])
                        dst_rows = stg[gi * DGRP:(gi + 1) * DGRP, :]
                        nc.vector.tensor_tensor(out=dst_rows, in0=tp[:],
                                                in1=ps_a[:],
                                                op=mybir.AluOpType.add)
                        if mode == "B":
                            nc.vector.tensor_scalar_max(dst_rows, dst_rows, 0.0)
                    if mode == "A":
                        ucast = stgpool.tile([P, P], bf16, tag="ucast")
                        nc.vector.tensor_copy(ucast[:], stg[:])
                        nc.sync.dma_start(u_own[t * P:(t + 1) * P, :], ucast[:])
                    else:
                        ptr = psC.tile([P, 2 * P], f32, tag="c")
                        nc.tensor.transpose(ptr[:, 0:P], stg[:], ident[:])
                        nc.scalar.copy(hT[:, t * P:(t + 1) * P], ptr[:, 0:P])
                        if do_pool:
                            nc.tensor.matmul(
                                pool_ps[:],
                                lhsT=spool_t[:, t * W_G:(t + 1) * W_G],
                                rhs=stg[:],
                                start=(l == 1 and t == 0),
                                stop=(l == N_LAYERS - 1 and t == TILES - 1))

            def max_pool_layer():
                m1 = smpool.tile([P, GROUPS], f32, tag="m1")
                nc.vector.tensor_reduce(
                    out=m1[:], in_=hT[:].rearrange("p (g d) -> p g d", d=DGRP),
                    axis=mybir.AxisListType.X, op=mybir.AluOpType.max)
                for li in range(N_LOC):
                    mb = psC.tile([P, 2 * P], f32, tag="c")
                    nc.tensor.matmul(
                        mb[:, 0:GROUPS], lhsT=ones_col[:],
                        rhs=mrowP[:, li * GROUPS:(li + 1) * GROUPS],
                        start=True, stop=True)
                    msel = smpool.tile([P, GROUPS], f32, tag="msel")
                    nc.vector.tensor_tensor(out=msel[:], in0=m1[:],
                                            in1=mb[:, 0:GROUPS],
                                            op=mybir.AluOpType.mult)
                    gm = smpool.tile([P, 1], f32, tag="gm")
                    nc.vector.tensor_reduce(out=gm[:], in_=msel[:],
                                            axis=mybir.AxisListType.X,
                                            op=mybir.AluOpType.max)
                    nc.vector.tensor_tensor(out=gmax12[:, li:li + 1],
                                            in0=gmax12[:, li:li + 1], in1=gm[:],
                                            op=mybir.AluOpType.add)

            # ================ main layer loop
            for l in range(N_LAYERS):
                produce(l, c_own)
                if not SKIP_AG:
                    nc.gpsimd.collective_compute(
                        "AllGather", mybir.AluOpType.bypass,
                        ins=[c_own[:]], outs=[c_full[:]],
                        replica_groups=[list(range(NCORES))])
                prop_pass(l, c_full[:], "A", False)
                if not SKIP_AG:
                    nc.gpsimd.collective_compute(
                        "AllGather", mybir.AluOpType.bypass,
                        ins=[u_own[:]], outs=[u_full[:]],
                        replica_groups=[list(range(NCORES))])
                prop_pass(l, u_full[:], "B", l >= 1)
                if l >= 1:
                    max_pool_layer()

            # ================ pooled outputs -> DRAM -> AllReduce
            sums = smpool.tile([W_G, F_IN], f32, tag="sums")
            nc.scalar.copy(sums[:], pool_ps[:])
            nc.gpsimd.indirect_dma_start(
                out=sumbuf[:],
                out_offset=bass.IndirectOffsetOnAxis(ap=gidx_t[:, 0:1], axis=0),
                in_=sums[:], in_offset=None)
            pmx = psC.tile([P, 2 * P], f32, tag="c")
            nc.tensor.transpose(pmx[0:W_G, 0:P], gmax12[:], ident[:])
            mxs = smpool.tile([W_G, P], f32, tag="mxs")
            nc.scalar.copy(mxs[:], pmx[0:W_G, 0:P])
            nc.gpsimd.indirect_dma_start(
                out=maxbuf[:],
                out_offset=bass.IndirectOffsetOnAxis(ap=gidx_t[:, 0:1], axis=0),
                in_=mxs[:], in_offset=None)
            nc.gpsimd.collective_compute(
                "AllReduce", mybir.AluOpType.max,
                ins=[maxbuf[:]], outs=[maxbuf_o[:]],
                replica_groups=[list(range(NCORES))])
            nc.gpsimd.collective_compute(
                "AllReduce", mybir.AluOpType.add,
                ins=[sumbuf[:]], outs=[sumbuf_o[:]],
                replica_groups=[list(range(NCORES))])

            # ================ head (replicated)
            rT = smpool.tile([P, 4 * P], f32, tag="rT")  # [feat128, max256|mean256]
            for half in range(2):
                mx = smpool.tile([P, F_IN], f32, tag="mx")
                nc.sync.dma_start(mx[:], maxbuf_o[half * P:(half + 1) * P, :])
                sm = smpool.tile([P, F_IN], f32, tag="smh")
                nc.sync.dma_start(sm[:], sumbuf_o[half * P:(half + 1) * P, :])
                nc.vector.tensor_scalar(out=sm[:], in0=sm[:],
                                        scalar1=rcg_t[:, half:half + 1],
                                        scalar2=None, op0=mybir.AluOpType.mult)
                pmxT = psC.tile([P, 2 * P], f32, tag="c")
                nc.tensor.transpose(pmxT[:, 0:P], mx[:], ident[:])
                nc.scalar.copy(rT[:, half * P:(half + 1) * P], pmxT[:, 0:P])
                psmT = psC.tile([P, 2 * P], f32, tag="c")
                nc.tensor.transpose(psmT[:, 0:P], sm[:], ident[:])
                nc.scalar.copy(rT[:, 2 * P + half * P:2 * P + (half + 1) * P],
                               psmT[:, 0:P])

            z1p = psC.tile([P, 2 * P], f32, tag="c")
            nc.tensor.matmul(z1p[:F2, 0:2 * P], lhsT=l1w[:, 0:F2],
                             rhs=rT[:, 0:2 * P], start=True, stop=False)
            nc.tensor.matmul(z1p[:F2, 0:2 * P], lhsT=l1w[:, F2:2 * F2],
                             rhs=rT[:, 2 * P:4 * P], start=False, stop=True)
            z1 = smpool.tile([F2, 2 * P], f32, tag="z1")
            nc.scalar.activation(z1[:], z1p[:F2, 0:2 * P],
                                 mybir.ActivationFunctionType.Relu,
                                 bias=l1b[:, 0:1], scale=1.0)
            z2p = psC.tile([P, 2 * P], f32, tag="c")
            nc.tensor.matmul(z2p[:H_HID, 0:2 * P], lhsT=l2w[:], rhs=z1[:],
                             start=True, stop=True)
            z2 = smpool.tile([H_HID, 2 * P], f32, tag="z2")
            nc.scalar.activation(z2[:], z2p[:H_HID, 0:2 * P],
                                 mybir.ActivationFunctionType.Relu,
                                 bias=l2b[:, 0:1], scale=1.0)
            z3p = psC.tile([P, 2 * P], f32, tag="c")
            nc.tensor.matmul(z3p[:N_CLS, 0:2 * P], lhsT=l3w[:], rhs=z2[:],
                             start=True, stop=True)
            z3 = smpool.tile([N_CLS, 2 * P], f32, tag="z3")
            nc.scalar.activation(z3[:], z3p[:N_CLS, 0:2 * P],
                                 mybir.ActivationFunctionType.Identity,
                                 bias=l3b[:, 0:1], scale=1.0)
            for half in range(2):
                lg = psC.tile([P, 2 * P], f32, tag="c")
                nc.tensor.transpose(lg[:, 0:N_CLS],
                                    z3[:, half * P:(half + 1) * P],
                                    ident[0:N_CLS, 0:N_CLS])
                lgs = smpool.tile([P, N_CLS], f32, tag="lgs")
                nc.vector.tensor_copy(lgs[:], lg[:, 0:N_CLS])
                rmax = smpool.tile([P, 1], f32, tag="rmax")
                nc.vector.tensor_reduce(out=rmax[:], in_=lgs[:],
                                        axis=mybir.AxisListType.X,
                                        op=mybir.AluOpType.max)
                xm = smpool.tile([P, N_CLS], f32, tag="xm")
                nc.vector.tensor_scalar(out=xm[:], in0=lgs[:],
                                        scalar1=rmax[:, 0:1], scalar2=None,
                                        op0=mybir.AluOpType.subtract)
                ex = smpool.tile([P, N_CLS], f32, tag="ex")
                nc.scalar.activation(ex[:], xm[:],
                                     mybir.ActivationFunctionType.Exp)
                sume = smpool.tile([P, 1], f32, tag="sume")
                nc.vector.tensor_reduce(out=sume[:], in_=ex[:],
                                        axis=mybir.AxisListType.X,
                                        op=mybir.AluOpType.add)
                lse = smpool.tile([P, 1], f32, tag="lse")
                nc.scalar.activation(lse[:], sume[:],
                                     mybir.ActivationFunctionType.Ln)
                res = smpool.tile([P, N_CLS], f32, tag="res")
                nc.vector.tensor_scalar(out=res[:], in0=xm[:],
                                        scalar1=lse[:, 0:1], scalar2=None,
                                        op0=mybir.AluOpType.subtract)
                nc.sync.dma_start(out[half * P:(half + 1) * P, :], res[:])

    nc.compile()
    return nc


# ------------------------------------------------------------------ runner
def _make_runner(nc, n_cores):
    import jax
    import concourse.mybir as mybir
    from jax.experimental.shard_map import shard_map
    from jax.sharding import Mesh, NamedSharding, PartitionSpec
    from concourse.bass2jax import (_bass_exec_p, install_neuronx_cc_hook,
                                    partition_id_tensor)

    install_neuronx_cc_hook()
    partition_name = nc.partition_id_tensor.name if nc.partition_id_tensor else None
    in_names, out_names, out_avals = [], [], []
    for alloc in nc.m.functions[0].allocations:
        if not isinstance(alloc, mybir.MemoryLocationSet):
            continue
        name = alloc.memorylocations[0].name
        if alloc.kind == "ExternalInput":
            if name != partition_name:
                in_names.append(name)
        elif alloc.kind == "ExternalOutput":
            out_names.append(name)
            out_avals.append(jax.core.ShapedArray(
                tuple(alloc.tensor_shape), mybir.dt.np(alloc.dtype)))
    n_params = len(in_names)
    all_in = list(in_names) + list(out_names)
    if partition_name is not None:
        all_in.append(partition_name)

    def _body(*args):
        operands = list(args)
        if partition_name is not None:
            operands.append(partition_id_tensor())
        return tuple(_bass_exec_p.bind(
            *operands, out_avals=tuple(out_avals), in_names=tuple(all_in),
            out_names=tuple(out_names), lowering_input_output_aliases=(),
            sim_require_finite=False, sim_require_nnan=False, nc=nc))

    devices = jax.devices()[:n_cores]
    mesh = Mesh(np.asarray(devices), ("core",))
    nin = n_params + len(out_names)
    sharded = jax.jit(shard_map(
        _body, mesh=mesh, in_specs=(PartitionSpec("core"),) * nin,
        out_specs=(PartitionSpec("core"),) * len(out_names), check_rep=False),
        keep_unused=True)
    sharding = NamedSharding(mesh, PartitionSpec("core"))

    def stage(in_maps):
        import jax as _jax
        concat_in = [np.concatenate([np.asarray(in_maps[c][nm])
                                     for c in range(n_cores)], axis=0)
                     for nm in in_names]
        concat_zero = [np.zeros((n_cores * a.shape[0], *a.shape[1:]), a.dtype)
                       for a in out_avals]
        return [_jax.device_put(x, sharding) for x in concat_in + concat_zero]

    def call(staged):
        import jax as _jax
        outs = sharded(*staged)
        _jax.block_until_ready(outs)
        return outs

    def fetch(outs):
        return [{nm: np.asarray(outs[i]).reshape(n_cores, *out_avals[i].shape)[c]
                 for i, nm in enumerate(out_names)} for c in range(n_cores)]

    return stage, call, fetch


_CACHE = {}


def _get_compiled(st):
    key = (st["S_core"], st["C_CH"], tuple(st["cpg"].tolist()))
    if key not in _CACHE:
        nc = _build(st)
        _CACHE[key] = (nc, _make_runner(nc, NCORES))
    return _CACHE[key]


def _in_maps(st, per_core, x, inputs):
    node_slot = st["node_slot"]
    S_core = st["S_core"]
    hom_W = np.asarray(inputs["hom_W"], np.float32)
    het_W = np.asarray(inputs["het_W"], np.float32)
    hom_b = np.asarray(inputs["hom_b"], np.float32)
    het_b = np.asarray(inputs["het_b"], np.float32)
    w2 = np.ascontiguousarray(np.concatenate([hom_W[:, 2], het_W[:, 2]], axis=2))
    w1 = np.ascontiguousarray(np.concatenate([hom_W[:, 1], het_W[:, 1]], axis=2))
    w0 = np.ascontiguousarray(np.concatenate([hom_W[:, 0], het_W[:, 0]], axis=2))
    bb = np.ascontiguousarray(np.concatenate([hom_b, het_b], axis=1)[:, None, :])
    rcount = np.zeros((NGP, 1), np.float32)
    rcount[:N_GRAPHS, 0] = 1.0 / np.maximum(st["counts"], 1.0)
    consts = _host_consts(st)

    x = np.asarray(x, np.float32)
    maps = []
    for c in range(NCORES):
        xo = np.zeros((S_core, F_IN), np.float32)
        m = (node_slot >= c * S_core) & (node_slot < (c + 1) * S_core)
        xo[node_slot[m] - c * S_core] = x[m]
        pc = per_core[c]
        maps.append({
            "x_own": xo, "idx16": pc["idx16"], "dlA": pc["dl"],
            "mqA": pc["mq"], "rdegA": pc["rdeg"], "batchloc": pc["batchloc"],
            "maskrow": pc["maskrow"], "gidx": pc["gidx"], "rcount_g": rcount,
            "iotaDG": consts["iotaDG"], "iotaWG": consts["iotaWG"],
            "identA": consts["identA"],
            "w2cat": w2, "w1cat": w1, "w0cat": w0, "bcat": bb,
            "lin1w": np.asarray(inputs["lin1_W"], np.float32),
            "lin1b": np.asarray(inputs["lin1_b"], np.float32)[:, None],
            "lin2w": np.asarray(inputs["lin2_W"], np.float32),
            "lin2b": np.asarray(inputs["lin2_b"], np.float32)[:, None],
            "lin3w": np.asarray(inputs["lin3_W"], np.float32),
            "lin3b": np.asarray(inputs["lin3_b"], np.float32)[:, None]})
    return maps


def kernel(**inputs):
    x = np.asarray(inputs["x"])
    edge_index = np.asarray(inputs["edge_index"])
    batch = np.asarray(inputs["batch"])
    st, per_core = _prep(edge_index, batch, inputs["hom_mask"], inputs["het_mask"])
    nc, (stage, call, fetch) = _get_compiled(st)
    maps = _in_maps(st, per_core, x, inputs)
    staged = stage(maps)
    outs = call(staged)
    return fetch(outs)[0]["out"].astype(np.float32)


# revision 24
# speedup vs baseline: 1.8257x; 1.8257x over previous
"""BiViewMixHop GNN kernel for 8 Trainium2 NeuronCores (Bass/Tile).

Strategy (v3):
  - Algebraic restructure: P(h)@W1 + P^2(h)@W2 = P(h@W1 + P(h@W2)); hom/het
    views fused into one 128-col tensor -> 2 gather passes per layer (6 total).
  - Host prep (index manipulation only): relabel nodes into graph-aligned
    32-slot-padded "slots", shard whole graphs contiguously across 8 cores,
    sort each core's edges by dst slot, pad each 32-slot group's edge list to
    a multiple of 128 ("chunks"). Chunk counts per group are maxed across
    cores so ONE SPMD program serves all 8 cores. Pad positions point at
    pseudo-random table rows (a single hot row serializes HBM).
  - Gather: batched SWDGE dma_gather (mlp ucode), 1024 indices per
    instruction (the SWDGE throughput sweet spot), 4 queues round-robin,
    issued with lookahead from the consume loop. Tables (c/u) are bf16,
    PAIR-PACKED: one 512B row = two consecutive slots' features, so int16
    indices cover the 54k-slot space.
  - Scatter/segment-sum: per chunk, a one-hot "eq" matrix (dstloc == iota)
    is the PE stationary operand; per-edge hom/het masks (split by slot
    parity to pick the right half of each gathered pair) are multiplied into
    the gathered tile on DVE (broadcast tensor_tensor, one op per gather
    batch, emitted with lookahead). Two N=128 matmuls per chunk accumulate
    even+odd message sums into one PSUM tile.
  - Degree normalization: 1/deg precomputed on host, applied as per-partition
    ACT scale at PSUM evacuation.
  - Cross-core exchange: each gather table is filled by TWO half AllGathers
    (bf16); the first half's collective is issued mid-pass as soon as the
    producing tiles are written, hiding it under the remaining compute.
    produce (c = h@W2) is fused per-tile into pass B so AG(c) starts early.
  - Readout: mean-pool via PE matmuls against one-hot graph selectors;
    max-pool via windowed reduce + per-graph mask+reduce; cross-core combine
    via ONE AllGather + on-chip max/add reduce; MLP head + log_softmax
    replicated per core.
"""

import numpy as np

# ---------------------------------------------------------------- constants
F_IN = 128
H_HID = 64
N_LAYERS = 3
N_GRAPHS = 256
N_CLS = 10
NCORES = 8
P = 128
DGRP = 32           # dst-group width (eq-matrix columns)
SBATCH = 16         # chunks per eq-build batch
BCH = 8             # chunks per dma_gather batch (1024 idxs: SWDGE sweet spot)
W_G = 64            # per-core local-graph window
NQ = 4              # SWDGE queues
NGP = N_GRAPHS + 32  # pooled rows (graphs + pad; 2*NGP = 64*9 for the reduce)
SKIP_GATHER = False  # timing knockout: memset instead of dma_gather
SKIP_AG = False      # timing knockout: skip AllGathers


# ------------------------------------------------------------------ host prep
def _prep(edge_index, batch, hom_mask, het_mask):
    """Index-only preprocessing. Returns structure dict + per-core arrays."""
    import ml_dtypes
    bf16 = ml_dtypes.bfloat16

    N = batch.shape[0]
    src, dst = np.asarray(edge_index[0]), np.asarray(edge_index[1])
    batch = np.asarray(batch)
    counts = np.bincount(batch, minlength=N_GRAPHS)

    pad_sizes = ((counts + DGRP - 1) // DGRP) * DGRP
    total_slots = int(pad_sizes.sum())
    target = max(total_slots / NCORES, 1.0)

    csum = np.concatenate([[0], np.cumsum(pad_sizes)])
    gcore = np.minimum((csum[:-1] + pad_sizes / 2) / target, NCORES - 1).astype(np.int64)
    gcore = np.maximum.accumulate(gcore)

    core_slots = np.zeros(NCORES, np.int64)
    graph_base = np.zeros(N_GRAPHS, np.int64)
    for g in range(N_GRAPHS):
        c = gcore[g]
        graph_base[g] = core_slots[c]
        core_slots[c] += pad_sizes[g]
    S_core = int(((core_slots.max() + P - 1) // P) * P)
    TILES = S_core // P
    GROUPS = S_core // DGRP
    TOT = NCORES * S_core
    assert TOT % 2 == 0 and TOT // 2 < 32768, "pair-packed idx must fit int16"
    HALF_T = (TILES + 1) // 2   # tiles in region A (AG split point)
    HS = HALF_T * P

    node_starts = np.concatenate([[0], np.cumsum(counts)])
    node_slot = np.empty(N, np.int64)
    for g in range(N_GRAPHS):
        a, b = node_starts[g], node_starts[g + 1]
        if b > a:
            base = gcore[g] * S_core + graph_base[g]
            node_slot[a:b] = base + np.arange(b - a)

    src_slot = node_slot[src]
    dst_slot = node_slot[dst]
    owner = dst_slot // S_core

    core_graphs = [np.where((gcore == c) & (counts > 0))[0] for c in range(NCORES)]
    n_loc = max(len(cg) for cg in core_graphs)
    assert n_loc <= W_G, f"{n_loc} local graphs > W_G={W_G}"

    cpg = np.ones(GROUPS, np.int64)
    per_core_edges = []
    hom_mask = np.asarray(hom_mask)
    het_mask = np.asarray(het_mask)
    for c in range(NCORES):
        m = owner == c
        es, ed = src_slot[m], dst_slot[m]
        eh, et = hom_mask[m], het_mask[m]
        order = np.argsort(ed, kind="stable")
        es, ed, eh, et = es[order], ed[order], eh[order], et[order]
        loc = ed - c * S_core
        grp = loc // DGRP
        gstart = np.searchsorted(grp, np.arange(GROUPS))
        gend = np.searchsorted(grp, np.arange(GROUPS) + 1)
        per_core_edges.append((es, loc, eh, et, gstart, gend))
        cnt = gend - gstart
        cpg = np.maximum(cpg, (cnt + P - 1) // P)
    C_CH = int(cpg.sum())
    cog = np.concatenate([[0], np.cumsum(cpg)])
    NIDX = C_CH * P
    IDXCOLS = NIDX // 16

    st = {"S_core": S_core, "TILES": TILES, "GROUPS": GROUPS, "TOT": TOT,
          "C_CH": C_CH, "cpg": cpg, "cog": cog, "node_slot": node_slot,
          "counts": counts, "n_loc": n_loc, "IDXCOLS": IDXCOLS,
          "HALF_T": HALF_T, "HS": HS}

    # slot -> pair-row in the split-region table layout:
    # region A = all cores' local slots [0:HS), region B = the rest.
    NAp = NCORES * HS // 2

    def slot_to_row(s):
        c = s // S_core
        ls = s % S_core
        rowA = (c * HS + ls) // 2
        rowB = NAp + (c * (S_core - HS) + (ls - HS)) // 2
        return np.where(ls < HS, rowA, rowB)

    per_core = []
    for c in range(NCORES):
        es, loc, eh, et, gstart, gend = per_core_edges[c]
        # pad positions must NOT all hit one table row (HBM hotspot):
        # spread them deterministically over the whole slot space
        spread = (np.arange(P * C_CH, dtype=np.int64) * 2654435761) % TOT
        srcA = spread.reshape(P, C_CH)
        dlA = np.zeros((P, C_CH), np.float32)
        mhA = np.zeros((P, C_CH), np.float32)
        mtA = np.zeros((P, C_CH), np.float32)
        for g in range(GROUPS):
            a, b = gstart[g], gend[g]
            c0 = cog[g]
            for j in range(cpg[g]):
                lo = a + j * P
                hi = min(a + (j + 1) * P, b)
                if hi <= lo:
                    break
                k = hi - lo
                srcA[:k, c0 + j] = es[lo:hi]
                dlA[:k, c0 + j] = (loc[lo:hi] - g * DGRP).astype(np.float32)
                mhA[:k, c0 + j] = eh[lo:hi]
                mtA[:k, c0 + j] = et[lo:hi]

        # int16 pair-row indices, wrapped into 16 partitions column-major and
        # replicated across the 8 Q7 cores' partition groups
        rows = slot_to_row(srcA)
        flat = rows.astype(np.int16).flatten(order="F")  # pos i=(p,j)
        idx16 = np.zeros((P, IDXCOLS), np.int16)
        base16 = flat.reshape(IDXCOLS, 16).T
        for k in range(8):
            idx16[16 * k:16 * (k + 1), :] = base16

        # per-edge masks split by src-slot parity, interleaved per chunk as
        # (even-hom, even-het, odd-hom, odd-het) quads
        par = (srcA % 2).astype(np.float32)
        mq = np.zeros((P, 4 * C_CH), np.float32)
        mq[:, 0::4] = mhA * (1.0 - par)
        mq[:, 1::4] = mtA * (1.0 - par)
        mq[:, 2::4] = mhA * par
        mq[:, 3::4] = mtA * par

        # 1/deg per dst slot, hom/het, laid out [128, 2*TILES]
        dst_all = loc
        rdeg = np.zeros((P, 2 * TILES), np.float32)
        dh = 1.0 / np.maximum(np.bincount(dst_all, weights=eh, minlength=S_core), 1.0)
        dt = 1.0 / np.maximum(np.bincount(dst_all, weights=et, minlength=S_core), 1.0)
        rdeg[:, 0::2] = dh.reshape(TILES, P).T
        rdeg[:, 1::2] = dt.reshape(TILES, P).T

        batchloc = np.full(S_core, -1.0, np.float32)
        maskrow = np.zeros((W_G, GROUPS), np.float32)
        gidx = np.full((W_G, 2), N_GRAPHS, np.int32)  # pads -> dump row
        gidx[:, 1] = N_GRAPHS + NGP
        for li, g in enumerate(core_graphs[c]):
            base = graph_base[g]
            batchloc[base:base + counts[g]] = li
            g0, g1 = base // DGRP, (base + pad_sizes[g]) // DGRP
            maskrow[li, g0:g1] = 1.0
            gidx[li, 0] = g            # max region row
            gidx[li, 1] = NGP + g      # sum region row
        per_core.append({
            "idx16": idx16,
            "dl": dlA.astype(bf16),
            "mq": mq.astype(bf16),
            "rdeg": rdeg,
            "batchloc": batchloc.reshape(TILES, P).T.copy(),
            "maskrow": maskrow.reshape(1, -1).copy(), "gidx": gidx})
    return st, per_core


def _host_consts(st):
    """Constant helper tensors (iota patterns, identity)."""
    import ml_dtypes
    bf16 = ml_dtypes.bfloat16
    iotaDG = np.tile(np.arange(DGRP, dtype=np.float32), SBATCH)[None, :].repeat(P, 0)
    iotaWG = np.arange(W_G, dtype=np.float32)[None, :].repeat(P, 0)
    ident = np.eye(P, dtype=np.float32)
    return {"iotaDG": iotaDG.astype(bf16), "iotaWG": iotaWG, "identA": ident}


# ------------------------------------------------------------- device builder
def _build(st):
    import concourse.bass as bass
    import concourse.bacc as bacc
    import concourse.mybir as mybir
    import concourse.tile as tile

    S_core, TILES, GROUPS = st["S_core"], st["TILES"], st["GROUPS"]
    TOT, C_CH, cog = st["TOT"], st["C_CH"], st["cog"]
    IDXCOLS, N_LOC = st["IDXCOLS"], st["n_loc"]
    HALF_T, HS = st["HALF_T"], st["HS"]
    NB = (C_CH + BCH - 1) // BCH      # gather batches per pass
    NSB = (C_CH + SBATCH - 1) // SBATCH
    NAp = NCORES * HS // 2            # pair rows in table region A
    SB = S_core - HS                  # region-B slots per core
    f32 = mybir.dt.float32
    bf16 = mybir.dt.bfloat16
    F2 = 2 * H_HID  # 128
    RGRP = 2 * NGP  # rows in fused readout buffer (max | sum)
    RB = 64         # readout reduce partition dim (split at 32 is aligned)
    assert RGRP % RB == 0
    RA = RGRP // RB

    nc = bacc.Bacc("TRN2", target_bir_lowering=False, debug=False,
                   num_devices=NCORES, num_swdge_queues=NQ)

    def din(name, shape, dtype=f32):
        return nc.dram_tensor(name, shape, dtype, kind="ExternalInput").ap()

    x_own = din("x_own", [S_core, F_IN])
    idx16 = din("idx16", [P, IDXCOLS], mybir.dt.int16)
    dlA = din("dlA", [P, C_CH], bf16)
    mqA = din("mqA", [P, 4 * C_CH], bf16)
    rdegA = din("rdegA", [P, 2 * TILES])
    batchloc = din("batchloc", [P, TILES])
    maskrow = din("maskrow", [1, W_G * GROUPS])
    gidx = din("gidx", [W_G, 2], mybir.dt.int32)
    rcount_g = din("rcount_g", [NGP, 1])
    iotaDG_d = din("iotaDG", [P, SBATCH * DGRP], bf16)
    iotaWG_d = din("iotaWG", [P, W_G])
    identA = din("identA", [P, P])
    w2cat = din("w2cat", [N_LAYERS, F_IN, F2])
    w1cat = din("w1cat", [N_LAYERS, F_IN, F2])
    w0cat = din("w0cat", [N_LAYERS, F_IN, F2])
    bcat = din("bcat", [N_LAYERS, 1, F2])
    lin1w = din("lin1w", [4 * H_HID, 2 * H_HID])
    lin1b = din("lin1b", [2 * H_HID, 1])
    lin2w = din("lin2w", [2 * H_HID, H_HID])
    lin2b = din("lin2b", [H_HID, 1])
    lin3w = din("lin3w", [H_HID, N_CLS])
    lin3b = din("lin3b", [N_CLS, 1])

    out = nc.dram_tensor("out", [N_GRAPHS, N_CLS], f32, kind="ExternalOutput").ap()

    c_own_a = nc.dram_tensor("c_own_a", [HS, F_IN], bf16).ap()
    c_own_b = nc.dram_tensor("c_own_b", [SB, F_IN], bf16).ap()
    u_own_a = nc.dram_tensor("u_own_a", [HS, F_IN], bf16).ap()
    u_own_b = nc.dram_tensor("u_own_b", [SB, F_IN], bf16).ap()
    # pair-packed gather tables: one row = two consecutive slots' features;
    # rows [0:NAp) = all cores' region A, rows [NAp:) = region B
    c_full = nc.dram_tensor("c_full", [TOT // 2, 2 * F_IN], bf16,
                            addr_space="Shared").ap()
    u_full = nc.dram_tensor("u_full", [TOT // 2, 2 * F_IN], bf16,
                            addr_space="Shared").ap()
    rbuf = nc.dram_tensor("rbuf", [RGRP, F_IN], f32).ap()
    rbuf_o = nc.dram_tensor("rbuf_o", [NCORES * RGRP, F_IN], f32,
                            addr_space="Shared").ap()
    maxfin = nc.dram_tensor("maxfin", [NGP, F_IN], f32).ap()
    sumfin = nc.dram_tensor("sumfin", [NGP, F_IN], f32).ap()

    with tile.TileContext(nc) as tc:
        with tc.tile_pool(name="const", bufs=1) as cpool, \
             tc.tile_pool(name="sb", bufs=3) as spool, \
             tc.tile_pool(name="gt", bufs=8) as gpool, \
             tc.tile_pool(name="stg", bufs=3) as stgpool, \
             tc.tile_pool(name="sm", bufs=3) as smpool, \
             tc.tile_pool(name="psA", bufs=3, space="PSUM") as psA, \
             tc.tile_pool(name="psB", bufs=2, space="PSUM") as psB, \
             tc.tile_pool(name="psC", bufs=2, space="PSUM") as psC, \
             tc.tile_pool(name="psP", bufs=1, space="PSUM") as psP:

            # ---------------- resident tiles
            ident = cpool.tile([P, P], f32)
            nc.sync.dma_start(ident[:], identA[:])
            idx_t = cpool.tile([P, IDXCOLS], mybir.dt.int16)
            nc.sync.dma_start(idx_t[:], idx16[:])
            dl_t = cpool.tile([P, C_CH], bf16)
            nc.sync.dma_start(dl_t[:], dlA[:])
            mq_t = cpool.tile([P, 4 * C_CH], bf16)
            nc.sync.dma_start(mq_t[:], mqA[:])
            iota32 = cpool.tile([P, SBATCH * DGRP], bf16)
            nc.sync.dma_start(iota32[:], iotaDG_d[:])
            iotaWG = cpool.tile([P, W_G], f32)
            nc.sync.dma_start(iotaWG[:], iotaWG_d[:])
            ones_col = cpool.tile([1, P], f32)
            nc.vector.memset(ones_col[:], 1.0)
            zero_t = cpool.tile([P, P], f32)
            nc.vector.memset(zero_t[:], 0.0)
            hT = cpool.tile([P, S_core], f32)
            rdeg = cpool.tile([P, 2 * TILES], f32)
            nc.sync.dma_start(rdeg[:], rdegA[:])
            bl_t = cpool.tile([P, TILES], f32)
            nc.sync.dma_start(bl_t[:], batchloc[:])
            mrowP = cpool.tile([1, N_LOC * GROUPS], f32)
            nc.sync.dma_start(mrowP[:], maskrow[:, 0:N_LOC * GROUPS])
            gidx_t = cpool.tile([W_G, 2], mybir.dt.int32)
            nc.sync.dma_start(gidx_t[:], gidx[:])
            rcg_t = cpool.tile([P, 2], f32)
            nc.sync.dma_start(
                rcg_t[:], rcount_g[:2 * P, :].rearrange("(a b) o -> b (a o)", a=2))
            wAll = cpool.tile([P, 9 * F2], f32)
            for l in range(N_LAYERS):
                nc.sync.dma_start(wAll[:, (3 * l + 0) * F2:(3 * l + 1) * F2], w2cat[l])
                nc.sync.dma_start(wAll[:, (3 * l + 1) * F2:(3 * l + 2) * F2], w1cat[l])
                nc.sync.dma_start(wAll[:, (3 * l + 2) * F2:(3 * l + 3) * F2], w0cat[l])
            bAll = cpool.tile([1, N_LAYERS * F2], f32)
            for l in range(N_LAYERS):
                nc.sync.dma_start(bAll[:, l * F2:(l + 1) * F2], bcat[l])
            l1w = cpool.tile([P, 2 * F2], f32)  # two K-halves side by side
            nc.sync.dma_start(l1w[:, 0:F2], lin1w[0:P, :])
            nc.sync.dma_start(l1w[:, F2:2 * F2], lin1w[P:2 * P, :])
            l2w = cpool.tile([2 * H_HID, H_HID], f32)
            nc.sync.dma_start(l2w[:], lin2w[:])
            l2b = cpool.tile([H_HID, 1], f32)
            nc.sync.dma_start(l2b[:], lin2b[:])
            l1b = cpool.tile([2 * H_HID, 1], f32)
            nc.sync.dma_start(l1b[:], lin1b[:])
            l3w = cpool.tile([H_HID, N_CLS], f32)
            nc.sync.dma_start(l3w[:], lin3w[:])
            l3b = cpool.tile([N_CLS, 1], f32)
            nc.sync.dma_start(l3b[:], lin3b[:])
            spool_t = cpool.tile([P, TILES * W_G], f32)
            gmax12 = cpool.tile([P, W_G], f32)
            nc.vector.memset(gmax12[:], 0.0)
            racc = cpool.tile([RB, RA * F_IN], f32)

            def wslice(l, which):  # 0=w2, 1=w1, 2=w0
                o = (3 * l + which) * F2
                return wAll[:, o:o + F2]

            # ---------------- x -> hT (feature-major)
            for t in range(TILES):
                xs = stgpool.tile([P, P], f32, tag="xs")
                nc.sync.dma_start(xs[:], x_own[t * P:(t + 1) * P, :])
                pt = psC.tile([P, 2 * P], f32, tag="c")
                nc.tensor.transpose(pt[:, 0:P], xs[:], ident[:])
                nc.scalar.copy(hT[:, t * P:(t + 1) * P], pt[:, 0:P])

            # ---------------- S_pool (one-hot local-graph selectors)
            for t in range(TILES):
                nc.vector.tensor_tensor(
                    out=spool_t[:, t * W_G:(t + 1) * W_G],
                    in0=bl_t[:, t:t + 1].to_broadcast([P, W_G]),
                    in1=iotaWG[:], op=mybir.AluOpType.is_equal)

            # ---------------- zero-fill fused readout buffer
            r = 0
            while r < RGRP:
                k = min(P, RGRP - r)
                nc.sync.dma_start(rbuf[r:r + k, :], zero_t[:k, :])
                r += k

            pool_ps = psP.tile([W_G, F_IN], f32)

            def ag_half(which, half):
                if SKIP_AG:
                    return
                own_a, own_b, full = (
                    (c_own_a, c_own_b, c_full) if which == "c"
                    else (u_own_a, u_own_b, u_full))
                if half == 0:
                    nc.gpsimd.collective_compute(
                        "AllGather", mybir.AluOpType.bypass,
                        ins=[own_a[:]], outs=[full[0:NAp, :]],
                        replica_groups=[list(range(NCORES))])
                else:
                    nc.gpsimd.collective_compute(
                        "AllGather", mybir.AluOpType.bypass,
                        ins=[own_b[:]], outs=[full[NAp:TOT // 2, :]],
                        replica_groups=[list(range(NCORES))])

            def emit_produce_tile(l, t):
                pc = psC.tile([P, 2 * P], f32, tag="c")
                nc.tensor.matmul(pc[:, 0:F2], lhsT=hT[:, t * P:(t + 1) * P],
                                 rhs=wslice(l, 0), start=True, stop=True)
                cs = stgpool.tile([P, F2], bf16, tag="cs")
                nc.scalar.copy(cs[:], pc[:, 0:F2])
                if t < HALF_T:
                    nc.sync.dma_start(c_own_a[t * P:(t + 1) * P, :], cs[:])
                else:
                    tb = t - HALF_T
                    nc.sync.dma_start(c_own_b[tb * P:(tb + 1) * P, :], cs[:])

            def produce0():
                for t in range(TILES):
                    emit_produce_tile(0, t)
                    if t == HALF_T - 1:
                        ag_half("c", 0)
                ag_half("c", 1)

            def build_eq(b):
                c0 = b * SBATCH
                nch = min(SBATCH, C_CH - c0)
                eq = spool.tile([P, SBATCH * DGRP], bf16, tag="eq")
                r3 = lambda ap: ap.rearrange("p (k d) -> p k d", d=DGRP)[:, :nch, :]
                nc.vector.tensor_tensor(
                    out=r3(eq[:]),
                    in0=dl_t[:, c0:c0 + nch, None].to_broadcast([P, nch, DGRP]),
                    in1=r3(iota32[:]), op=mybir.AluOpType.is_equal)
                return eq

            nreg_cache = {}

            def nreg(n):
                if n not in nreg_cache:
                    nreg_cache[n] = nc.gpsimd.to_reg(n)
                return nreg_cache[n]


            def prop_pass(l, table, mode, do_pool):
                # mode A: u = P(c) + h@W1 -> u_own halves + AG(u) halves
                # mode B: h = relu(P(u) + h@W0 + b) -> hT, optional fused
                #         produce(l+1) + AG(c) halves, optional pooling
                nxt = l + 1 if (mode == "B" and l + 1 < N_LAYERS) else None
                gts = [None] * NB
                issued = [-1]
                LA_GATHER = 6

                def ensure_issued(b):
                    tgt = min(b, NB - 1)
                    while issued[0] < tgt:
                        issued[0] += 1
                        bb = issued[0]
                        j0 = bb * BCH
                        nch = min(BCH, C_CH - j0)
                        gt = gpool.tile([P, BCH * 2 * F_IN], bf16, tag="g")
                        if SKIP_GATHER:
                            nc.vector.memset(gt[:, :nch * 2 * F_IN], 0.25)
                        else:
                            out3 = gt[:, :nch * 2 * F_IN].rearrange(
                                "p (c e) -> p c e", e=2 * F_IN)
                            nc.gpsimd.dma_gather(
                                out3, table, idx_t[:, j0 * 8:(j0 + nch) * 8],
                                nch * P, nreg(nch * P), 2 * F_IN,
                                queue_num=0, single_packet=False)
                        gts[bb] = gt

                def mask_mult(b):
                    j0 = b * BCH
                    nch = min(BCH, C_CH - j0)
                    gt = gts[b]
                    m4 = gt[:, :nch * 2 * F_IN].rearrange(
                        "p (c s) -> p c s", s=H_HID)
                    nc.vector.tensor_tensor(
                        out=m4,
                        in0=m4,
                        in1=mq_t[:, 4 * j0:4 * (j0 + nch), None].to_broadcast(
                            [P, 4 * nch, H_HID]),
                        op=mybir.AluOpType.mult)

                eq_cache = {}

                def get_eq(b):
                    if b not in eq_cache:
                        for k in (b, min(b + 1, NSB - 1)):
                            if k not in eq_cache:
                                eq_cache[k] = build_eq(k)
                        for k in [k for k in eq_cache if k < b - 1]:
                            del eq_cache[k]
                    return eq_cache[b]

                masked_upto = [-1]
                LA_MASK = 4  # < gpool bufs to stay deadlock-free

                def ensure_masked(b):
                    tgt = min(b + LA_MASK, NB - 1)
                    while masked_upto[0] < tgt:
                        masked_upto[0] += 1
                        mask_mult(masked_upto[0])

                for t in range(TILES):
                    stg = stgpool.tile([P, P], f32, tag="hstg")
                    for gi in range(4):
                        g = t * 4 + gi
                        ps_t = psA.tile([DGRP, F_IN], f32, tag="t")
                        chunks = list(range(cog[g], cog[g + 1]))
                        for j in chunks:
                            ensure_issued(j // BCH + LA_GATHER)
                            eq = get_eq(j // SBATCH)
                            jj = j % SBATCH
                            ensure_masked(j // BCH)
                            gt = gts[j // BCH]
                            jb = j % BCH
                            lhs = eq[:, jj * DGRP:(jj + 1) * DGRP]
                            nc.tensor.matmul(
                                ps_t[:],
                                lhsT=lhs,
                                rhs=gt[:, jb * 2 * F_IN:jb * 2 * F_IN + F_IN],
                                start=j == chunks[0], stop=False)
                            nc.tensor.matmul(
                                ps_t[:],
                                lhsT=lhs,
                                rhs=gt[:, jb * 2 * F_IN + F_IN:(jb + 1) * 2 * F_IN],
                                start=False, stop=j == chunks[-1])
                        ps_a = psB.tile([DGRP, F_IN], f32, tag="a")
                        nc.tensor.matmul(
                            ps_a[:], lhsT=hT[:, g * DGRP:(g + 1) * DGRP],
                            rhs=wslice(l, 1 if mode == "A" else 2),
                            start=True, stop=(mode == "A"))
                        if mode == "B":
                            nc.tensor.matmul(ps_a[:], lhsT=ones_col[:, 0:DGRP],
                                             rhs=bAll[:, l * F2:(l + 1) * F2],
                                             start=False, stop=True)
                        r0 = (g % 4) * DGRP
                        t0 = g // 4
                        tp = smpool.tile([DGRP, F_IN], f32, tag="tp")
                        nc.scalar.mul(tp[:, 0:H_HID], ps_t[:, 0:H_HID],
                                      rdeg[r0:r0 + DGRP, 2 * t0:2 * t0 + 1])
                        nc.scalar.mul(tp[:, H_HID:F_IN], ps_t[:, H_HID:F_IN],
                                      rdeg[r0:r0 + DGRP, 2 * t0 + 1:2 * t0 + 2])
                        dst_rows = stg[gi * DGRP:(gi + 1) * DGRP, :]
                        nc.vector.tensor_tensor(out=dst_rows, in0=tp[:],
                                                in1=ps_a[:],
                                                op=mybir.AluOpType.add)
                        if mode == "B":
                            nc.vector.tensor_scalar_max(dst_rows, dst_rows, 0.0)
                    if mode == "A":
                        ucast = stgpool.tile([P, P], bf16, tag="ucast")
                        nc.vector.tensor_copy(ucast[:], stg[:])
                        if t < HALF_T:
                            nc.sync.dma_start(u_own_a[t * P:(t + 1) * P, :],
                                              ucast[:])
                        else:
                            tb = t - HALF_T
                            nc.sync.dma_start(u_own_b[tb * P:(tb + 1) * P, :],
                                              ucast[:])
                        if t == HALF_T - 1:
                            ag_half("u", 0)
                    else:
                        ptr = psC.tile([P, 2 * P], f32, tag="c")
                        nc.tensor.transpose(ptr[:, 0:P], stg[:], ident[:])
                        nc.scalar.copy(hT[:, t * P:(t + 1) * P], ptr[:, 0:P])
                        if nxt is not None:
                            emit_produce_tile(nxt, t)
                            if t == HALF_T - 1:
                                ag_half("c", 0)
                        if do_pool:
                            nc.tensor.matmul(
                                pool_ps[:],
                                lhsT=spool_t[:, t * W_G:(t + 1) * W_G],
                                rhs=stg[:],
                                start=(l == 1 and t == 0),
                                stop=(l == N_LAYERS - 1 and t == TILES - 1))
                if mode == "A":
                    ag_half("u", 1)
                elif nxt is not None:
                    ag_half("c", 1)

            def max_pool_layer():
                m1 = smpool.tile([P, GROUPS], f32, tag="m1")
                nc.vector.tensor_reduce(
                    out=m1[:], in_=hT[:].rearrange("p (g d) -> p g d", d=DGRP),
                    axis=mybir.AxisListType.X, op=mybir.AluOpType.max)
                for li in range(N_LOC):
                    mb = psC.tile([P, 2 * P], f32, tag="c")
                    nc.tensor.matmul(
                        mb[:, 0:GROUPS], lhsT=ones_col[:],
                        rhs=mrowP[:, li * GROUPS:(li + 1) * GROUPS],
                        start=True, stop=True)
                    msel = smpool.tile([P, GROUPS], f32, tag="msel")
                    nc.vector.tensor_tensor(out=msel[:], in0=m1[:],
                                            in1=mb[:, 0:GROUPS],
                                            op=mybir.AluOpType.mult)
                    gm = smpool.tile([P, 1], f32, tag="gm")
                    nc.vector.tensor_reduce(out=gm[:], in_=msel[:],
                                            axis=mybir.AxisListType.X,
                                            op=mybir.AluOpType.max)
                    nc.vector.tensor_tensor(out=gmax12[:, li:li + 1],
                                            in0=gmax12[:, li:li + 1], in1=gm[:],
                                            op=mybir.AluOpType.add)

            # ================ main layer loop
            produce0()
            for l in range(N_LAYERS):
                prop_pass(l, c_full[:], "A", False)
                prop_pass(l, u_full[:], "B", l >= 1)
                if l >= 1:
                    max_pool_layer()

            # ================ pooled outputs -> rbuf -> AllGather -> reduce
            sums = smpool.tile([W_G, F_IN], f32, tag="sums")
            nc.scalar.copy(sums[:], pool_ps[:])
            nc.gpsimd.indirect_dma_start(
                out=rbuf[:],
                out_offset=bass.IndirectOffsetOnAxis(ap=gidx_t[:, 1:2], axis=0),
                in_=sums[:], in_offset=None)
            pmx = psC.tile([P, 2 * P], f32, tag="c")
            nc.tensor.transpose(pmx[0:W_G, 0:P], gmax12[:], ident[:])
            mxs = smpool.tile([W_G, P], f32, tag="mxs")
            nc.scalar.copy(mxs[:], pmx[0:W_G, 0:P])
            nc.gpsimd.indirect_dma_start(
                out=rbuf[:],
                out_offset=bass.IndirectOffsetOnAxis(ap=gidx_t[:, 0:1], axis=0),
                in_=mxs[:], in_offset=None)
            nc.gpsimd.collective_compute(
                "AllGather", mybir.AluOpType.bypass,
                ins=[rbuf[:]], outs=[rbuf_o[:]],
                replica_groups=[list(range(NCORES))])
            # reduce across the 8 core blocks: rows [0:NGP)=max, [NGP:)=sum.
            # Block layout trick: view each block [RGRP,128] as [RB, 8*128];
            # row r = b*8+a so b<RB/2 is exactly the max region.
            for k in range(NCORES):
                blk = smpool.tile([RB, RA * F_IN], f32, tag="blk")
                nc.sync.dma_start(
                    blk[:], rbuf_o[k * RGRP:(k + 1) * RGRP, :].rearrange(
                        "(b a) f -> b (a f)", b=RB))
                if k == 0:
                    nc.vector.tensor_copy(racc[:], blk[:])
                else:
                    nc.vector.tensor_tensor(
                        out=racc[0:RB // 2, :], in0=racc[0:RB // 2, :],
                        in1=blk[0:RB // 2, :], op=mybir.AluOpType.max)
                    nc.vector.tensor_tensor(
                        out=racc[RB // 2:RB, :], in0=racc[RB // 2:RB, :],
                        in1=blk[RB // 2:RB, :], op=mybir.AluOpType.add)
            nc.sync.dma_start(
                maxfin[:].rearrange("(b a) f -> b (a f)", b=RB // 2),
                racc[0:RB // 2, :])
            nc.sync.dma_start(
                sumfin[:].rearrange("(b a) f -> b (a f)", b=RB // 2),
                racc[RB // 2:RB, :])

            # ================ head (replicated)
            rT = smpool.tile([P, 4 * P], f32, tag="rT")  # [feat128, max256|mean256]
            for half in range(2):
                mx = smpool.tile([P, F_IN], f32, tag="mx")
                nc.sync.dma_start(mx[:], maxfin[half * P:(half + 1) * P, :])
                sm = smpool.tile([P, F_IN], f32, tag="smh")
                nc.sync.dma_start(sm[:], sumfin[half * P:(half + 1) * P, :])
                nc.vector.tensor_scalar(out=sm[:], in0=sm[:],
                                        scalar1=rcg_t[:, half:half + 1],
                                        scalar2=None, op0=mybir.AluOpType.mult)
                pmxT = psC.tile([P, 2 * P], f32, tag="c")
                nc.tensor.transpose(pmxT[:, 0:P], mx[:], ident[:])
                nc.scalar.copy(rT[:, half * P:(half + 1) * P], pmxT[:, 0:P])
                psmT = psC.tile([P, 2 * P], f32, tag="c")
                nc.tensor.transpose(psmT[:, 0:P], sm[:], ident[:])
                nc.scalar.copy(rT[:, 2 * P + half * P:2 * P + (half + 1) * P],
                               psmT[:, 0:P])

            z1p = psC.tile([P, 2 * P], f32, tag="c")
            nc.tensor.matmul(z1p[:F2, 0:2 * P], lhsT=l1w[:, 0:F2],
                             rhs=rT[:, 0:2 * P], start=True, stop=False)
            nc.tensor.matmul(z1p[:F2, 0:2 * P], lhsT=l1w[:, F2:2 * F2],
                             rhs=rT[:, 2 * P:4 * P], start=False, stop=True)
            z1 = smpool.tile([F2, 2 * P], f32, tag="z1")
            nc.scalar.activation(z1[:], z1p[:F2, 0:2 * P],
                                 mybir.ActivationFunctionType.Relu,
                                 bias=l1b[:, 0:1], scale=1.0)
            z2p = psC.tile([P, 2 * P], f32, tag="c")
            nc.tensor.matmul(z2p[:H_HID, 0:2 * P], lhsT=l2w[:], rhs=z1[:],
                             start=True, stop=True)
            z2 = smpool.tile([H_HID, 2 * P], f32, tag="z2")
            nc.scalar.activation(z2[:], z2p[:H_HID, 0:2 * P],
                                 mybir.ActivationFunctionType.Relu,
                                 bias=l2b[:, 0:1], scale=1.0)
            z3p = psC.tile([P, 2 * P], f32, tag="c")
            nc.tensor.matmul(z3p[:N_CLS, 0:2 * P], lhsT=l3w[:], rhs=z2[:],
                             start=True, stop=True)
            z3 = smpool.tile([N_CLS, 2 * P], f32, tag="z3")
            nc.scalar.activation(z3[:], z3p[:N_CLS, 0:2 * P],
                                 mybir.ActivationFunctionType.Identity,
                                 bias=l3b[:, 0:1], scale=1.0)
            for half in range(2):
                lg = psC.tile([P, 2 * P], f32, tag="c")
                nc.tensor.transpose(lg[:, 0:N_CLS],
                                    z3[:, half * P:(half + 1) * P],
                                    ident[0:N_CLS, 0:N_CLS])
                lgs = smpool.tile([P, N_CLS], f32, tag="lgs")
                nc.vector.tensor_copy(lgs[:], lg[:, 0:N_CLS])
                rmax = smpool.tile([P, 1], f32, tag="rmax")
                nc.vector.tensor_reduce(out=rmax[:], in_=lgs[:],
                                        axis=mybir.AxisListType.X,
                                        op=mybir.AluOpType.max)
                xm = smpool.tile([P, N_CLS], f32, tag="xm")
                nc.vector.tensor_scalar(out=xm[:], in0=lgs[:],
                                        scalar1=rmax[:, 0:1], scalar2=None,
                                        op0=mybir.AluOpType.subtract)
                ex = smpool.tile([P, N_CLS], f32, tag="ex")
                nc.scalar.activation(ex[:], xm[:],
                                     mybir.ActivationFunctionType.Exp)
                sume = smpool.tile([P, 1], f32, tag="sume")
                nc.vector.tensor_reduce(out=sume[:], in_=ex[:],
                                        axis=mybir.AxisListType.X,
                                        op=mybir.AluOpType.add)
                lse = smpool.tile([P, 1], f32, tag="lse")
                nc.scalar.activation(lse[:], sume[:],
                                     mybir.ActivationFunctionType.Ln)
                res = smpool.tile([P, N_CLS], f32, tag="res")
                nc.vector.tensor_scalar(out=res[:], in0=xm[:],
                                        scalar1=lse[:, 0:1], scalar2=None,
                                        op0=mybir.AluOpType.subtract)
                nc.sync.dma_start(out[half * P:(half + 1) * P, :], res[:])

    nc.compile()

    # Post-schedule queue realignment: Tile assigns SWDGE completion sems
    # round-robin over 8 DMASW lanes in SCHEDULED order. A lane must always
    # serve the same hw queue, else cross-queue completion reordering can
    # satisfy a cumulative wait threshold before an earlier same-lane DMA
    # has landed. Pin each lane's queue to lane%NQ -- except lanes that host
    # an InstDMACopy (indirect scatter), which always executes on queue 0.
    pool_dmas = []
    for blk in nc.m.functions[0].blocks:
        for inst in blk.instructions:
            if inst.engine != mybir.EngineType.Pool:
                continue
            tname = type(inst).__name__
            if tname not in ("InstDMAGatherAnt", "InstDMACopy"):
                continue
            lane = None
            si = inst.sync_info
            if si:
                for u in si.on_update:
                    if u.ant_name and u.ant_name.startswith("DMASW"):
                        lane = int(u.ant_name[5:].split("_")[0])
            if lane is not None:
                pool_dmas.append((inst, tname, lane))
    lane_q = {lane: lane % NQ for _, _, lane in pool_dmas}
    for _, tname, lane in pool_dmas:
        if tname == "InstDMACopy":
            lane_q[lane] = 0
    for inst, tname, lane in pool_dmas:
        if tname == "InstDMAGatherAnt":
            inst.queue_num = lane_q[lane]
        else:
            inst.queue = "qPoolDynamic"
    return nc


# ------------------------------------------------------------------ runner
def _make_runner(nc, n_cores):
    import jax
    import concourse.mybir as mybir
    from jax.experimental.shard_map import shard_map
    from jax.sharding import Mesh, NamedSharding, PartitionSpec
    from concourse.bass2jax import (_bass_exec_p, install_neuronx_cc_hook,
                                    partition_id_tensor)

    install_neuronx_cc_hook()
    partition_name = nc.partition_id_tensor.name if nc.partition_id_tensor else None
    in_names, out_names, out_avals = [], [], []
    for alloc in nc.m.functions[0].allocations:
        if not isinstance(alloc, mybir.MemoryLocationSet):
            continue
        name = alloc.memorylocations[0].name
        if alloc.kind == "ExternalInput":
            if name != partition_name:
                in_names.append(name)
        elif alloc.kind == "ExternalOutput":
            out_names.append(name)
            out_avals.append(jax.core.ShapedArray(
                tuple(alloc.tensor_shape), mybir.dt.np(alloc.dtype)))
    n_params = len(in_names)
    all_in = list(in_names) + list(out_names)
    if partition_name is not None:
        all_in.append(partition_name)

    def _body(*args):
        operands = list(args)
        if partition_name is not None:
            operands.append(partition_id_tensor())
        return tuple(_bass_exec_p.bind(
            *operands, out_avals=tuple(out_avals), in_names=tuple(all_in),
            out_names=tuple(out_names), lowering_input_output_aliases=(),
            sim_require_finite=False, sim_require_nnan=False, nc=nc))

    devices = jax.devices()[:n_cores]
    mesh = Mesh(np.asarray(devices), ("core",))
    nin = n_params + len(out_names)
    sharded = jax.jit(shard_map(
        _body, mesh=mesh, in_specs=(PartitionSpec("core"),) * nin,
        out_specs=(PartitionSpec("core"),) * len(out_names), check_rep=False),
        keep_unused=True)
    sharding = NamedSharding(mesh, PartitionSpec("core"))

    def stage(in_maps):
        import jax as _jax
        concat_in = [np.concatenate([np.asarray(in_maps[c][nm])
                                     for c in range(n_cores)], axis=0)
                     for nm in in_names]
        concat_zero = [np.zeros((n_cores * a.shape[0], *a.shape[1:]), a.dtype)
                       for a in out_avals]
        return [_jax.device_put(x, sharding) for x in concat_in + concat_zero]

    def call(staged):
        import jax as _jax
        outs = sharded(*staged)
        _jax.block_until_ready(outs)
        return outs

    def fetch(outs):
        return [{nm: np.asarray(outs[i]).reshape(n_cores, *out_avals[i].shape)[c]
                 for i, nm in enumerate(out_names)} for c in range(n_cores)]

    return stage, call, fetch


_CACHE = {}


def _get_compiled(st):
    key = (st["S_core"], st["C_CH"], tuple(st["cpg"].tolist()))
    if key not in _CACHE:
        nc = _build(st)
        _CACHE[key] = (nc, _make_runner(nc, NCORES))
    return _CACHE[key]


def _in_maps(st, per_core, x, inputs):
    node_slot = st["node_slot"]
    S_core = st["S_core"]
    hom_W = np.asarray(inputs["hom_W"], np.float32)
    het_W = np.asarray(inputs["het_W"], np.float32)
    hom_b = np.asarray(inputs["hom_b"], np.float32)
    het_b = np.asarray(inputs["het_b"], np.float32)
    w2 = np.ascontiguousarray(np.concatenate([hom_W[:, 2], het_W[:, 2]], axis=2))
    w1 = np.ascontiguousarray(np.concatenate([hom_W[:, 1], het_W[:, 1]], axis=2))
    w0 = np.ascontiguousarray(np.concatenate([hom_W[:, 0], het_W[:, 0]], axis=2))
    bb = np.ascontiguousarray(np.concatenate([hom_b, het_b], axis=1)[:, None, :])
    rcount = np.zeros((NGP, 1), np.float32)
    rcount[:N_GRAPHS, 0] = 1.0 / np.maximum(st["counts"], 1.0)
    consts = _host_consts(st)

    x = np.asarray(x, np.float32)
    maps = []
    for c in range(NCORES):
        xo = np.zeros((S_core, F_IN), np.float32)
        m = (node_slot >= c * S_core) & (node_slot < (c + 1) * S_core)
        xo[node_slot[m] - c * S_core] = x[m]
        pc = per_core[c]
        maps.append({
            "x_own": xo, "idx16": pc["idx16"], "dlA": pc["dl"],
            "mqA": pc["mq"], "rdegA": pc["rdeg"], "batchloc": pc["batchloc"],
            "maskrow": pc["maskrow"], "gidx": pc["gidx"], "rcount_g": rcount,
            "iotaDG": consts["iotaDG"], "iotaWG": consts["iotaWG"],
            "identA": consts["identA"],
            "w2cat": w2, "w1cat": w1, "w0cat": w0, "bcat": bb,
            "lin1w": np.asarray(inputs["lin1_W"], np.float32),
            "lin1b": np.asarray(inputs["lin1_b"], np.float32)[:, None],
            "lin2w": np.asarray(inputs["lin2_W"], np.float32),
            "lin2b": np.asarray(inputs["lin2_b"], np.float32)[:, None],
            "lin3w": np.asarray(inputs["lin3_W"], np.float32),
            "lin3b": np.asarray(inputs["lin3_b"], np.float32)[:, None]})
    return maps


def kernel(**inputs):
    x = np.asarray(inputs["x"])
    edge_index = np.asarray(inputs["edge_index"])
    batch = np.asarray(inputs["batch"])
    st, per_core = _prep(edge_index, batch, inputs["hom_mask"], inputs["het_mask"])
    nc, (stage, call, fetch) = _get_compiled(st)
    maps = _in_maps(st, per_core, x, inputs)
    staged = stage(maps)
    outs = call(staged)
    return fetch(outs)[0]["out"].astype(np.float32)


# revision 26
# speedup vs baseline: 2.6466x; 1.4496x over previous
"""BiViewMixHop GNN kernel for 8 Trainium2 NeuronCores (Bass/Tile).

Strategy (v3):
  - Algebraic restructure: P(h)@W1 + P^2(h)@W2 = P(h@W1 + P(h@W2)); hom/het
    views fused into one 128-col tensor -> 2 gather passes per layer (6 total).
  - Host prep (index manipulation only): relabel nodes into graph-aligned
    32-slot-padded "slots", shard whole graphs contiguously across 8 cores,
    sort each core's edges by dst slot, pad each 32-slot group's edge list to
    a multiple of 128 ("chunks"). Chunk counts per group are maxed across
    cores so ONE SPMD program serves all 8 cores. Pad positions point at
    pseudo-random table rows (a single hot row serializes HBM).
  - Gather: batched SWDGE dma_gather (mlp ucode), 1024 indices per
    instruction (the SWDGE throughput sweet spot), 4 queues round-robin,
    issued with lookahead from the consume loop. Tables (c/u) are bf16,
    PAIR-PACKED: one 512B row = two consecutive slots' features, so int16
    indices cover the 54k-slot space.
  - Scatter/segment-sum: per chunk, a one-hot "eq" matrix (dstloc == iota)
    is the PE stationary operand; per-edge hom/het masks (split by slot
    parity to pick the right half of each gathered pair) are multiplied into
    the gathered tile on DVE (broadcast tensor_tensor, one op per gather
    batch, emitted with lookahead). Two N=128 matmuls per chunk accumulate
    even+odd message sums into one PSUM tile.
  - Degree normalization: 1/deg precomputed on host, applied as per-partition
    ACT scale at PSUM evacuation.
  - Cross-core exchange: each gather table is filled by TWO half AllGathers
    (bf16); the first half's collective is issued mid-pass as soon as the
    producing tiles are written, hiding it under the remaining compute.
    produce (c = h@W2) is fused per-tile into pass B so AG(c) starts early.
  - Readout: mean-pool via PE matmuls against one-hot graph selectors;
    max-pool via windowed reduce + per-graph mask+reduce; cross-core combine
    via ONE AllGather + on-chip max/add reduce; MLP head + log_softmax
    replicated per core.
"""

import numpy as np

# ---------------------------------------------------------------- constants
F_IN = 128
H_HID = 64
N_LAYERS = 3
N_GRAPHS = 256
N_CLS = 10
NCORES = 8
P = 128
DGRP = 32           # dst-group width (eq-matrix columns)
SBATCH = 16         # chunks per eq-build batch
BCH = 8             # chunks per dma_gather batch (1024 idxs: SWDGE sweet spot)
W_G = 64            # per-core local-graph window
NQ = 4              # SWDGE queues
NGP = N_GRAPHS + 32  # pooled rows (graphs + pad; 2*NGP = 64*9 for the reduce)
SKIP_GATHER = False  # timing knockout: memset instead of dma_gather
SKIP_AG = False      # timing knockout: skip AllGathers


# ------------------------------------------------------------------ host prep
def _prep(edge_index, batch, hom_mask, het_mask):
    """Index-only preprocessing. Returns structure dict + per-core arrays."""
    import ml_dtypes
    bf16 = ml_dtypes.bfloat16

    N = batch.shape[0]
    src, dst = np.asarray(edge_index[0]), np.asarray(edge_index[1])
    batch = np.asarray(batch)
    counts = np.bincount(batch, minlength=N_GRAPHS)

    pad_sizes = ((counts + DGRP - 1) // DGRP) * DGRP
    total_slots = int(pad_sizes.sum())
    target = max(total_slots / NCORES, 1.0)

    csum = np.concatenate([[0], np.cumsum(pad_sizes)])
    gcore = np.minimum((csum[:-1] + pad_sizes / 2) / target, NCORES - 1).astype(np.int64)
    gcore = np.maximum.accumulate(gcore)

    core_slots = np.zeros(NCORES, np.int64)
    graph_base = np.zeros(N_GRAPHS, np.int64)
    for g in range(N_GRAPHS):
        c = gcore[g]
        graph_base[g] = core_slots[c]
        core_slots[c] += pad_sizes[g]
    S_core = int(((core_slots.max() + P - 1) // P) * P)
    TILES = S_core // P
    GROUPS = S_core // DGRP
    TOT = NCORES * S_core
    assert TOT % 2 == 0 and TOT // 2 < 32768, "pair-packed idx must fit int16"
    HALF_T = (TILES + 1) // 2   # tiles in region A (AG split point)
    HS = HALF_T * P

    node_starts = np.concatenate([[0], np.cumsum(counts)])
    node_slot = np.empty(N, np.int64)
    for g in range(N_GRAPHS):
        a, b = node_starts[g], node_starts[g + 1]
        if b > a:
            base = gcore[g] * S_core + graph_base[g]
            node_slot[a:b] = base + np.arange(b - a)

    src_slot = node_slot[src]
    dst_slot = node_slot[dst]
    owner = dst_slot // S_core

    core_graphs = [np.where((gcore == c) & (counts > 0))[0] for c in range(NCORES)]
    n_loc = max(len(cg) for cg in core_graphs)
    assert n_loc <= W_G, f"{n_loc} local graphs > W_G={W_G}"

    cpg = np.ones(GROUPS, np.int64)
    per_core_edges = []
    hom_mask = np.asarray(hom_mask)
    het_mask = np.asarray(het_mask)
    for c in range(NCORES):
        m = owner == c
        es, ed = src_slot[m], dst_slot[m]
        eh, et = hom_mask[m], het_mask[m]
        order = np.argsort(ed, kind="stable")
        es, ed, eh, et = es[order], ed[order], eh[order], et[order]
        loc = ed - c * S_core
        grp = loc // DGRP
        gstart = np.searchsorted(grp, np.arange(GROUPS))
        gend = np.searchsorted(grp, np.arange(GROUPS) + 1)
        per_core_edges.append((es, loc, eh, et, gstart, gend))
        cnt = gend - gstart
        cpg = np.maximum(cpg, (cnt + P - 1) // P)
    C_CH = int(cpg.sum())
    cog = np.concatenate([[0], np.cumsum(cpg)])
    NIDX = C_CH * P
    IDXCOLS = NIDX // 16

    st = {"S_core": S_core, "TILES": TILES, "GROUPS": GROUPS, "TOT": TOT,
          "C_CH": C_CH, "cpg": cpg, "cog": cog, "node_slot": node_slot,
          "counts": counts, "n_loc": n_loc, "IDXCOLS": IDXCOLS,
          "HALF_T": HALF_T, "HS": HS}

    # slot -> pair-row in the split-region table layout:
    # region A = all cores' local slots [0:HS), region B = the rest.
    NAp = NCORES * HS // 2

    def slot_to_row(s):
        c = s // S_core
        ls = s % S_core
        rowA = (c * HS + ls) // 2
        rowB = NAp + (c * (S_core - HS) + (ls - HS)) // 2
        return np.where(ls < HS, rowA, rowB)

    per_core = []
    for c in range(NCORES):
        es, loc, eh, et, gstart, gend = per_core_edges[c]
        # pad positions must NOT all hit one table row (HBM hotspot):
        # spread them deterministically over the whole slot space
        spread = (np.arange(P * C_CH, dtype=np.int64) * 2654435761) % TOT
        srcA = spread.reshape(P, C_CH)
        dlA = np.zeros((P, C_CH), np.float32)
        mhA = np.zeros((P, C_CH), np.float32)
        mtA = np.zeros((P, C_CH), np.float32)
        for g in range(GROUPS):
            a, b = gstart[g], gend[g]
            c0 = cog[g]
            for j in range(cpg[g]):
                lo = a + j * P
                hi = min(a + (j + 1) * P, b)
                if hi <= lo:
                    break
                k = hi - lo
                srcA[:k, c0 + j] = es[lo:hi]
                dlA[:k, c0 + j] = (loc[lo:hi] - g * DGRP).astype(np.float32)
                mhA[:k, c0 + j] = eh[lo:hi]
                mtA[:k, c0 + j] = et[lo:hi]

        # int16 pair-row indices, wrapped into 16 partitions column-major and
        # replicated across the 8 Q7 cores' partition groups
        rows = slot_to_row(srcA)
        flat = rows.astype(np.int16).flatten(order="F")  # pos i=(p,j)
        idx16 = np.zeros((P, IDXCOLS), np.int16)
        base16 = flat.reshape(IDXCOLS, 16).T
        for k in range(8):
            idx16[16 * k:16 * (k + 1), :] = base16

        # per-edge masks split by src-slot parity, interleaved per chunk as
        # (even-hom, even-het, odd-hom, odd-het) quads
        par = (srcA % 2).astype(np.float32)
        mq = np.zeros((P, 4 * C_CH), np.float32)
        mq[:, 0::4] = mhA * (1.0 - par)
        mq[:, 1::4] = mtA * (1.0 - par)
        mq[:, 2::4] = mhA * par
        mq[:, 3::4] = mtA * par

        # 1/deg per dst slot, hom/het, laid out [128, 2*TILES]
        dst_all = loc
        rdeg = np.zeros((P, 2 * TILES), np.float32)
        dh = 1.0 / np.maximum(np.bincount(dst_all, weights=eh, minlength=S_core), 1.0)
        dt = 1.0 / np.maximum(np.bincount(dst_all, weights=et, minlength=S_core), 1.0)
        rdeg[:, 0::2] = dh.reshape(TILES, P).T
        rdeg[:, 1::2] = dt.reshape(TILES, P).T

        batchloc = np.full(S_core, -1.0, np.float32)
        maskrow = np.zeros((W_G, GROUPS), np.float32)
        gidx = np.full((W_G, 2), N_GRAPHS, np.int32)  # pads -> dump row
        gidx[:, 1] = N_GRAPHS + NGP
        for li, g in enumerate(core_graphs[c]):
            base = graph_base[g]
            batchloc[base:base + counts[g]] = li
            g0, g1 = base // DGRP, (base + pad_sizes[g]) // DGRP
            maskrow[li, g0:g1] = 1.0
            gidx[li, 0] = g            # max region row
            gidx[li, 1] = NGP + g      # sum region row
        per_core.append({
            "idx16": idx16,
            "dl": dlA.astype(bf16),
            "mq": mq.astype(bf16),
            "rdeg": rdeg,
            "batchloc": batchloc.reshape(TILES, P).T.copy(),
            "maskrow": maskrow.reshape(1, -1).copy(), "gidx": gidx})
    return st, per_core


def _host_consts(st):
    """Constant helper tensors (iota patterns, identity)."""
    import ml_dtypes
    bf16 = ml_dtypes.bfloat16
    iotaDG = np.tile(np.arange(DGRP, dtype=np.float32), SBATCH)[None, :].repeat(P, 0)
    iotaWG = np.arange(W_G, dtype=np.float32)[None, :].repeat(P, 0)
    ident = np.eye(P, dtype=np.float32)
    return {"iotaDG": iotaDG.astype(bf16), "iotaWG": iotaWG, "identA": ident}


# ------------------------------------------------------------- device builder
def _build(st):
    import concourse.bass as bass
    import concourse.bacc as bacc
    import concourse.mybir as mybir
    import concourse.tile as tile

    S_core, TILES, GROUPS = st["S_core"], st["TILES"], st["GROUPS"]
    TOT, C_CH, cog = st["TOT"], st["C_CH"], st["cog"]
    IDXCOLS, N_LOC = st["IDXCOLS"], st["n_loc"]
    HALF_T, HS = st["HALF_T"], st["HS"]
    NB = (C_CH + BCH - 1) // BCH      # gather batches per pass
    NSB = (C_CH + SBATCH - 1) // SBATCH
    NAp = NCORES * HS // 2            # pair rows in table region A
    SB = S_core - HS                  # region-B slots per core
    f32 = mybir.dt.float32
    bf16 = mybir.dt.bfloat16
    F2 = 2 * H_HID  # 128
    RGRP = 2 * NGP  # rows in fused readout buffer (max | sum)
    RB = 64         # readout reduce partition dim (split at 32 is aligned)
    assert RGRP % RB == 0
    RA = RGRP // RB

    nc = bacc.Bacc("TRN2", target_bir_lowering=False, debug=False,
                   num_devices=NCORES, num_swdge_queues=NQ)

    def din(name, shape, dtype=f32):
        return nc.dram_tensor(name, shape, dtype, kind="ExternalInput").ap()

    x_own = din("x_own", [F_IN, S_core])
    idx16 = din("idx16", [P, IDXCOLS], mybir.dt.int16)
    dlA = din("dlA", [P, C_CH], bf16)
    mqA = din("mqA", [P, 4 * C_CH], bf16)
    rdegA = din("rdegA", [P, 2 * TILES])
    batchloc = din("batchloc", [P, TILES])
    maskrow = din("maskrow", [1, W_G * GROUPS])
    gidx = din("gidx", [W_G, 2], mybir.dt.int32)
    rcount_g = din("rcount_g", [NGP, 1])
    iotaDG_d = din("iotaDG", [P, SBATCH * DGRP], bf16)
    iotaWG_d = din("iotaWG", [P, W_G])
    identA = din("identA", [P, P])
    w2cat = din("w2cat", [N_LAYERS, F_IN, F2])
    w1cat = din("w1cat", [N_LAYERS, F_IN, F2])
    w0cat = din("w0cat", [N_LAYERS, F_IN, F2])
    bcat = din("bcat", [N_LAYERS, 1, F2])
    lin1w = din("lin1w", [4 * H_HID, 2 * H_HID])
    lin1b = din("lin1b", [2 * H_HID, 1])
    lin2w = din("lin2w", [2 * H_HID, H_HID])
    lin2b = din("lin2b", [H_HID, 1])
    lin3w = din("lin3w", [H_HID, N_CLS])
    lin3b = din("lin3b", [N_CLS, 1])

    out = nc.dram_tensor("out", [N_GRAPHS, N_CLS], f32, kind="ExternalOutput").ap()

    c_own_a = nc.dram_tensor("c_own_a", [HS, F_IN], bf16).ap()
    c_own_b = nc.dram_tensor("c_own_b", [SB, F_IN], bf16).ap()
    u_own_a = nc.dram_tensor("u_own_a", [HS, F_IN], bf16).ap()
    u_own_b = nc.dram_tensor("u_own_b", [SB, F_IN], bf16).ap()
    # pair-packed gather tables: one row = two consecutive slots' features;
    # rows [0:NAp) = all cores' region A, rows [NAp:) = region B
    c_full = nc.dram_tensor("c_full", [TOT // 2, 2 * F_IN], bf16,
                            addr_space="Shared").ap()
    u_full = nc.dram_tensor("u_full", [TOT // 2, 2 * F_IN], bf16,
                            addr_space="Shared").ap()
    rbuf = nc.dram_tensor("rbuf", [RGRP, F_IN], f32).ap()
    rbuf_o = nc.dram_tensor("rbuf_o", [NCORES * RGRP, F_IN], f32,
                            addr_space="Shared").ap()
    maxfin = nc.dram_tensor("maxfin", [NGP, F_IN], f32).ap()
    sumfin = nc.dram_tensor("sumfin", [NGP, F_IN], f32).ap()

    with tile.TileContext(nc) as tc:
        with tc.tile_pool(name="const", bufs=1) as cpool, \
             tc.tile_pool(name="sb", bufs=3) as spool, \
             tc.tile_pool(name="gt", bufs=8) as gpool, \
             tc.tile_pool(name="stg", bufs=3) as stgpool, \
             tc.tile_pool(name="sm", bufs=3) as smpool, \
             tc.tile_pool(name="psA", bufs=3, space="PSUM") as psA, \
             tc.tile_pool(name="psB", bufs=2, space="PSUM") as psB, \
             tc.tile_pool(name="psC", bufs=2, space="PSUM") as psC, \
             tc.tile_pool(name="psP", bufs=1, space="PSUM") as psP:

            # ---------------- resident tiles
            ident = cpool.tile([P, P], f32)
            nc.sync.dma_start(ident[:], identA[:])
            idx_t = cpool.tile([P, IDXCOLS], mybir.dt.int16)
            nc.sync.dma_start(idx_t[:], idx16[:])
            dl_t = cpool.tile([P, C_CH], bf16)
            nc.sync.dma_start(dl_t[:], dlA[:])
            mq_t = cpool.tile([P, 4 * C_CH], bf16)
            nc.sync.dma_start(mq_t[:], mqA[:])
            iota32 = cpool.tile([P, SBATCH * DGRP], bf16)
            nc.sync.dma_start(iota32[:], iotaDG_d[:])
            iotaWG = cpool.tile([P, W_G], f32)
            nc.sync.dma_start(iotaWG[:], iotaWG_d[:])
            ones_col = cpool.tile([1, P], f32)
            nc.vector.memset(ones_col[:], 1.0)
            ones_bf = cpool.tile([1, P], bf16)
            nc.vector.memset(ones_bf[:], 1.0)
            zero_t = cpool.tile([P, P], f32)
            nc.vector.memset(zero_t[:], 0.0)
            hT = cpool.tile([P, S_core], f32)
            hT_bf = cpool.tile([P, S_core], bf16)
            rdeg = cpool.tile([P, 2 * TILES], f32)
            nc.sync.dma_start(rdeg[:], rdegA[:])
            bl_t = cpool.tile([P, TILES], f32)
            nc.sync.dma_start(bl_t[:], batchloc[:])
            mrowP = cpool.tile([1, N_LOC * GROUPS], f32)
            nc.sync.dma_start(mrowP[:], maskrow[:, 0:N_LOC * GROUPS])
            gidx_t = cpool.tile([W_G, 2], mybir.dt.int32)
            nc.sync.dma_start(gidx_t[:], gidx[:])
            rcg_t = cpool.tile([P, 2], f32)
            nc.sync.dma_start(
                rcg_t[:], rcount_g[:2 * P, :].rearrange("(a b) o -> b (a o)", a=2))
            wAll = cpool.tile([P, 9 * F2], f32)
            for l in range(N_LAYERS):
                nc.sync.dma_start(wAll[:, (3 * l + 0) * F2:(3 * l + 1) * F2], w2cat[l])
                nc.sync.dma_start(wAll[:, (3 * l + 1) * F2:(3 * l + 2) * F2], w1cat[l])
                nc.sync.dma_start(wAll[:, (3 * l + 2) * F2:(3 * l + 3) * F2], w0cat[l])
            bAll = cpool.tile([1, N_LAYERS * F2], f32)
            for l in range(N_LAYERS):
                nc.sync.dma_start(bAll[:, l * F2:(l + 1) * F2], bcat[l])
            bAll_bf = cpool.tile([1, N_LAYERS * F2], bf16)
            nc.vector.tensor_copy(bAll_bf[:], bAll[:])
            l1w = cpool.tile([P, 2 * F2], f32)  # two K-halves side by side
            nc.sync.dma_start(l1w[:, 0:F2], lin1w[0:P, :])
            nc.sync.dma_start(l1w[:, F2:2 * F2], lin1w[P:2 * P, :])
            l2w = cpool.tile([2 * H_HID, H_HID], f32)
            nc.sync.dma_start(l2w[:], lin2w[:])
            l2b = cpool.tile([H_HID, 1], f32)
            nc.sync.dma_start(l2b[:], lin2b[:])
            l1b = cpool.tile([2 * H_HID, 1], f32)
            nc.sync.dma_start(l1b[:], lin1b[:])
            l3w = cpool.tile([H_HID, N_CLS], f32)
            nc.sync.dma_start(l3w[:], lin3w[:])
            l3b = cpool.tile([N_CLS, 1], f32)
            nc.sync.dma_start(l3b[:], lin3b[:])
            spool_t = cpool.tile([P, TILES * W_G], f32)
            gmax12 = cpool.tile([P, W_G], f32)
            nc.vector.memset(gmax12[:], 0.0)
            racc = cpool.tile([RB, RA * F_IN], f32)

            wAll_bf = cpool.tile([P, 9 * F2], bf16)
            nc.vector.tensor_copy(wAll_bf[:], wAll[:])

            def wslice(l, which):  # 0=w2, 1=w1, 2=w0
                o = (3 * l + which) * F2
                return wAll_bf[:, o:o + F2]

            # ---------------- x -> hT (host-pretransposed, one DMA)
            nc.sync.dma_start(hT[:], x_own[:])
            nc.vector.tensor_copy(hT_bf[:], hT[:])

            # ---------------- S_pool (one-hot local-graph selectors)
            for t in range(TILES):
                nc.vector.tensor_tensor(
                    out=spool_t[:, t * W_G:(t + 1) * W_G],
                    in0=bl_t[:, t:t + 1].to_broadcast([P, W_G]),
                    in1=iotaWG[:], op=mybir.AluOpType.is_equal)

            if SKIP_AG:
                # knockout mode: fill gather tables once so reads are finite
                zb = cpool.tile([P, 2 * F_IN], bf16)
                nc.vector.memset(zb[:], 0.125)
                rr = 0
                while rr < TOT // 2:
                    kk = min(P, TOT // 2 - rr)
                    nc.sync.dma_start(c_full[rr:rr + kk, :], zb[:kk, :])
                    nc.sync.dma_start(u_full[rr:rr + kk, :], zb[:kk, :])
                    rr += P

            # ---------------- zero-fill fused readout buffer
            r = 0
            while r < RGRP:
                k = min(P, RGRP - r)
                nc.sync.dma_start(rbuf[r:r + k, :], zero_t[:k, :])
                r += k

            pool_ps = psP.tile([W_G, F_IN], f32)

            def ag_half(which, half):
                if SKIP_AG:
                    return
                own_a, own_b, full = (
                    (c_own_a, c_own_b, c_full) if which == "c"
                    else (u_own_a, u_own_b, u_full))
                if half == 0:
                    nc.gpsimd.collective_compute(
                        "AllGather", mybir.AluOpType.bypass,
                        ins=[own_a[:]], outs=[full[0:NAp, :]],
                        replica_groups=[list(range(NCORES))])
                else:
                    nc.gpsimd.collective_compute(
                        "AllGather", mybir.AluOpType.bypass,
                        ins=[own_b[:]], outs=[full[NAp:TOT // 2, :]],
                        replica_groups=[list(range(NCORES))])

            def emit_produce_tile(l, t):
                pc = psC.tile([P, 2 * P], f32, tag="c")
                nc.tensor.matmul(pc[:, 0:F2], lhsT=hT_bf[:, t * P:(t + 1) * P],
                                 rhs=wslice(l, 0), start=True, stop=True)
                cs = stgpool.tile([P, F2], bf16, tag="cs")
                nc.scalar.copy(cs[:], pc[:, 0:F2])
                if t < HALF_T:
                    nc.sync.dma_start(c_own_a[t * P:(t + 1) * P, :], cs[:])
                else:
                    tb = t - HALF_T
                    nc.sync.dma_start(c_own_b[tb * P:(tb + 1) * P, :], cs[:])

            def produce0():
                for t in range(TILES):
                    emit_produce_tile(0, t)
                    if t == HALF_T - 1:
                        ag_half("c", 0)
                ag_half("c", 1)

            def build_eq(b):
                c0 = b * SBATCH
                nch = min(SBATCH, C_CH - c0)
                eq = spool.tile([P, SBATCH * DGRP], bf16, tag="eq")
                r3 = lambda ap: ap.rearrange("p (k d) -> p k d", d=DGRP)[:, :nch, :]
                nc.vector.tensor_tensor(
                    out=r3(eq[:]),
                    in0=dl_t[:, c0:c0 + nch, None].to_broadcast([P, nch, DGRP]),
                    in1=r3(iota32[:]), op=mybir.AluOpType.is_equal)
                return eq

            nreg_cache = {}

            def nreg(n):
                if n not in nreg_cache:
                    nreg_cache[n] = nc.gpsimd.to_reg(n)
                return nreg_cache[n]


            def prop_pass(l, table, mode, do_pool):
                # mode A: u = P(c) + h@W1 -> u_own halves + AG(u) halves
                # mode B: h = relu(P(u) + h@W0 + b) -> hT, optional fused
                #         produce(l+1) + AG(c) halves, optional pooling
                nxt = l + 1 if (mode == "B" and l + 1 < N_LAYERS) else None
                gts = [None] * NB
                issued = [-1]
                LA_GATHER = 6

                def ensure_issued(b):
                    tgt = min(b, NB - 1)
                    while issued[0] < tgt:
                        issued[0] += 1
                        bb = issued[0]
                        j0 = bb * BCH
                        nch = min(BCH, C_CH - j0)
                        gt = gpool.tile([P, BCH * 2 * F_IN], bf16, tag="g")
                        if SKIP_GATHER:
                            nc.vector.memset(gt[:, :nch * 2 * F_IN], 0.25)
                        else:
                            out3 = gt[:, :nch * 2 * F_IN].rearrange(
                                "p (c e) -> p c e", e=2 * F_IN)
                            nc.gpsimd.dma_gather(
                                out3, table, idx_t[:, j0 * 8:(j0 + nch) * 8],
                                nch * P, nreg(nch * P), 2 * F_IN,
                                queue_num=0, single_packet=False)
                        gts[bb] = gt

                def mask_mult(b):
                    j0 = b * BCH
                    nch = min(BCH, C_CH - j0)
                    gt = gts[b]
                    m4 = gt[:, :nch * 2 * F_IN].rearrange(
                        "p (c s) -> p c s", s=H_HID)
                    nc.vector.tensor_tensor(
                        out=m4,
                        in0=m4,
                        in1=mq_t[:, 4 * j0:4 * (j0 + nch), None].to_broadcast(
                            [P, 4 * nch, H_HID]),
                        op=mybir.AluOpType.mult)

                eq_cache = {}

                def get_eq(b):
                    if b not in eq_cache:
                        for k in (b, min(b + 1, NSB - 1)):
                            if k not in eq_cache:
                                eq_cache[k] = build_eq(k)
                        for k in [k for k in eq_cache if k < b - 1]:
                            del eq_cache[k]
                    return eq_cache[b]

                masked_upto = [-1]
                LA_MASK = 4  # < gpool bufs to stay deadlock-free

                def ensure_masked(b):
                    tgt = min(b + LA_MASK, NB - 1)
                    while masked_upto[0] < tgt:
                        masked_upto[0] += 1
                        mask_mult(masked_upto[0])

                for t in range(TILES):
                    stg = stgpool.tile([P, P], f32, tag="hstg")
                    for gi in range(4):
                        g = t * 4 + gi
                        ps_t = psA.tile([DGRP, F_IN], f32, tag="t")
                        chunks = list(range(cog[g], cog[g + 1]))
                        for j in chunks:
                            ensure_issued(j // BCH + LA_GATHER)
                            eq = get_eq(j // SBATCH)
                            jj = j % SBATCH
                            ensure_masked(j // BCH)
                            gt = gts[j // BCH]
                            jb = j % BCH
                            lhs = eq[:, jj * DGRP:(jj + 1) * DGRP]
                            nc.tensor.matmul(
                                ps_t[:],
                                lhsT=lhs,
                                rhs=gt[:, jb * 2 * F_IN:jb * 2 * F_IN + F_IN],
                                start=j == chunks[0], stop=False)
                            nc.tensor.matmul(
                                ps_t[:],
                                lhsT=lhs,
                                rhs=gt[:, jb * 2 * F_IN + F_IN:(jb + 1) * 2 * F_IN],
                                start=False, stop=j == chunks[-1])
                        ps_a = psB.tile([DGRP, F_IN], f32, tag="a")
                        nc.tensor.matmul(
                            ps_a[:], lhsT=hT_bf[:, g * DGRP:(g + 1) * DGRP],
                            rhs=wslice(l, 1 if mode == "A" else 2),
                            start=True, stop=(mode == "A"))
                        if mode == "B":
                            nc.tensor.matmul(ps_a[:], lhsT=ones_bf[:, 0:DGRP],
                                             rhs=bAll_bf[:, l * F2:(l + 1) * F2],
                                             start=False, stop=True)
                        r0 = (g % 4) * DGRP
                        t0 = g // 4
                        tp = smpool.tile([DGRP, F_IN], f32, tag="tp")
                        nc.scalar.mul(tp[:, 0:H_HID], ps_t[:, 0:H_HID],
                                      rdeg[r0:r0 + DGRP, 2 * t0:2 * t0 + 1])
                        nc.scalar.mul(tp[:, H_HID:F_IN], ps_t[:, H_HID:F_IN],
                                      rdeg[r0:r0 + DGRP, 2 * t0 + 1:2 * t0 + 2])
                        dst_rows = stg[gi * DGRP:(gi + 1) * DGRP, :]
                        nc.vector.tensor_tensor(out=dst_rows, in0=tp[:],
                                                in1=ps_a[:],
                                                op=mybir.AluOpType.add)
                        if mode == "B":
                            nc.vector.tensor_scalar_max(dst_rows, dst_rows, 0.0)
                    if mode == "A":
                        ucast = stgpool.tile([P, P], bf16, tag="ucast")
                        nc.vector.tensor_copy(ucast[:], stg[:])
                        if t < HALF_T:
                            nc.sync.dma_start(u_own_a[t * P:(t + 1) * P, :],
                                              ucast[:])
                        else:
                            tb = t - HALF_T
                            nc.sync.dma_start(u_own_b[tb * P:(tb + 1) * P, :],
                                              ucast[:])
                        if t == HALF_T - 1:
                            ag_half("u", 0)
                    else:
                        ptr = psC.tile([P, 2 * P], f32, tag="c")
                        nc.tensor.transpose(ptr[:, 0:P], stg[:], ident[:])
                        nc.scalar.copy(hT[:, t * P:(t + 1) * P], ptr[:, 0:P])
                        nc.vector.tensor_copy(hT_bf[:, t * P:(t + 1) * P],
                                              hT[:, t * P:(t + 1) * P])
                        if nxt is not None:
                            emit_produce_tile(nxt, t)
                            if t == HALF_T - 1:
                                ag_half("c", 0)
                        if do_pool:
                            nc.tensor.matmul(
                                pool_ps[:],
                                lhsT=spool_t[:, t * W_G:(t + 1) * W_G],
                                rhs=stg[:],
                                start=(l == 1 and t == 0),
                                stop=(l == N_LAYERS - 1 and t == TILES - 1))
                if mode == "A":
                    ag_half("u", 1)
                elif nxt is not None:
                    ag_half("c", 1)

            def max_pool_layer():
                m1 = smpool.tile([P, GROUPS], f32, tag="m1")
                nc.vector.tensor_reduce(
                    out=m1[:], in_=hT[:].rearrange("p (g d) -> p g d", d=DGRP),
                    axis=mybir.AxisListType.X, op=mybir.AluOpType.max)
                for li in range(N_LOC):
                    mb = psC.tile([P, 2 * P], f32, tag="c")
                    nc.tensor.matmul(
                        mb[:, 0:GROUPS], lhsT=ones_col[:],
                        rhs=mrowP[:, li * GROUPS:(li + 1) * GROUPS],
                        start=True, stop=True)
                    msel = smpool.tile([P, GROUPS], f32, tag="msel")
                    nc.vector.tensor_tensor(out=msel[:], in0=m1[:],
                                            in1=mb[:, 0:GROUPS],
                                            op=mybir.AluOpType.mult)
                    gm = smpool.tile([P, 1], f32, tag="gm")
                    nc.vector.tensor_reduce(out=gm[:], in_=msel[:],
                                            axis=mybir.AxisListType.X,
                                            op=mybir.AluOpType.max)
                    nc.vector.tensor_tensor(out=gmax12[:, li:li + 1],
                                            in0=gmax12[:, li:li + 1], in1=gm[:],
                                            op=mybir.AluOpType.add)

            # ================ main layer loop
            produce0()
            for l in range(N_LAYERS):
                prop_pass(l, c_full[:], "A", False)
                prop_pass(l, u_full[:], "B", l >= 1)
                if l >= 1:
                    max_pool_layer()

            # ================ pooled outputs -> rbuf -> AllGather -> reduce
            sums = smpool.tile([W_G, F_IN], f32, tag="sums")
            nc.scalar.copy(sums[:], pool_ps[:])
            nc.gpsimd.indirect_dma_start(
                out=rbuf[:],
                out_offset=bass.IndirectOffsetOnAxis(ap=gidx_t[:, 1:2], axis=0),
                in_=sums[:], in_offset=None)
            pmx = psC.tile([P, 2 * P], f32, tag="c")
            nc.tensor.transpose(pmx[0:W_G, 0:P], gmax12[:], ident[:])
            mxs = smpool.tile([W_G, P], f32, tag="mxs")
            nc.scalar.copy(mxs[:], pmx[0:W_G, 0:P])
            nc.gpsimd.indirect_dma_start(
                out=rbuf[:],
                out_offset=bass.IndirectOffsetOnAxis(ap=gidx_t[:, 0:1], axis=0),
                in_=mxs[:], in_offset=None)
            nc.gpsimd.collective_compute(
                "AllGather", mybir.AluOpType.bypass,
                ins=[rbuf[:]], outs=[rbuf_o[:]],
                replica_groups=[list(range(NCORES))])
            # reduce across the 8 core blocks: rows [0:NGP)=max, [NGP:)=sum.
            # Block layout trick: view each block [RGRP,128] as [RB, 8*128];
            # row r = b*8+a so b<RB/2 is exactly the max region.
            for k in range(NCORES):
                blk = smpool.tile([RB, RA * F_IN], f32, tag="blk")
                nc.sync.dma_start(
                    blk[:], rbuf_o[k * RGRP:(k + 1) * RGRP, :].rearrange(
                        "(b a) f -> b (a f)", b=RB))
                if k == 0:
                    nc.vector.tensor_copy(racc[:], blk[:])
                else:
                    nc.vector.tensor_tensor(
                        out=racc[0:RB // 2, :], in0=racc[0:RB // 2, :],
                        in1=blk[0:RB // 2, :], op=mybir.AluOpType.max)
                    nc.vector.tensor_tensor(
                        out=racc[RB // 2:RB, :], in0=racc[RB // 2:RB, :],
                        in1=blk[RB // 2:RB, :], op=mybir.AluOpType.add)
            nc.sync.dma_start(
                maxfin[:].rearrange("(b a) f -> b (a f)", b=RB // 2),
                racc[0:RB // 2, :])
            nc.sync.dma_start(
                sumfin[:].rearrange("(b a) f -> b (a f)", b=RB // 2),
                racc[RB // 2:RB, :])

            # ================ head (replicated)
            rT = smpool.tile([P, 4 * P], f32, tag="rT")  # [feat128, max256|mean256]
            for half in range(2):
                mx = smpool.tile([P, F_IN], f32, tag="mx")
                nc.sync.dma_start(mx[:], maxfin[half * P:(half + 1) * P, :])
                sm = smpool.tile([P, F_IN], f32, tag="smh")
                nc.sync.dma_start(sm[:], sumfin[half * P:(half + 1) * P, :])
                nc.vector.tensor_scalar(out=sm[:], in0=sm[:],
                                        scalar1=rcg_t[:, half:half + 1],
                                        scalar2=None, op0=mybir.AluOpType.mult)
                pmxT = psC.tile([P, 2 * P], f32, tag="c")
                nc.tensor.transpose(pmxT[:, 0:P], mx[:], ident[:])
                nc.scalar.copy(rT[:, half * P:(half + 1) * P], pmxT[:, 0:P])
                psmT = psC.tile([P, 2 * P], f32, tag="c")
                nc.tensor.transpose(psmT[:, 0:P], sm[:], ident[:])
                nc.scalar.copy(rT[:, 2 * P + half * P:2 * P + (half + 1) * P],
                               psmT[:, 0:P])

            z1p = psC.tile([P, 2 * P], f32, tag="c")
            nc.tensor.matmul(z1p[:F2, 0:2 * P], lhsT=l1w[:, 0:F2],
                             rhs=rT[:, 0:2 * P], start=True, stop=False)
            nc.tensor.matmul(z1p[:F2, 0:2 * P], lhsT=l1w[:, F2:2 * F2],
                             rhs=rT[:, 2 * P:4 * P], start=False, stop=True)
            z1 = smpool.tile([F2, 2 * P], f32, tag="z1")
            nc.scalar.activation(z1[:], z1p[:F2, 0:2 * P],
                                 mybir.ActivationFunctionType.Relu,
                                 bias=l1b[:, 0:1], scale=1.0)
            z2p = psC.tile([P, 2 * P], f32, tag="c")
            nc.tensor.matmul(z2p[:H_HID, 0:2 * P], lhsT=l2w[:], rhs=z1[:],
                             start=True, stop=True)
            z2 = smpool.tile([H_HID, 2 * P], f32, tag="z2")
            nc.scalar.activation(z2[:], z2p[:H_HID, 0:2 * P],
                                 mybir.ActivationFunctionType.Relu,
                                 bias=l2b[:, 0:1], scale=1.0)
            z3p = psC.tile([P, 2 * P], f32, tag="c")
            nc.tensor.matmul(z3p[:N_CLS, 0:2 * P], lhsT=l3w[:], rhs=z2[:],
                             start=True, stop=True)
            z3 = smpool.tile([N_CLS, 2 * P], f32, tag="z3")
            nc.scalar.activation(z3[:], z3p[:N_CLS, 0:2 * P],
                                 mybir.ActivationFunctionType.Identity,
                                 bias=l3b[:, 0:1], scale=1.0)
            for half in range(2):
                lg = psC.tile([P, 2 * P], f32, tag="c")
                nc.tensor.transpose(lg[:, 0:N_CLS],
                                    z3[:, half * P:(half + 1) * P],
                                    ident[0:N_CLS, 0:N_CLS])
                lgs = smpool.tile([P, N_CLS], f32, tag="lgs")
                nc.vector.tensor_copy(lgs[:], lg[:, 0:N_CLS])
                rmax = smpool.tile([P, 1], f32, tag="rmax")
                nc.vector.tensor_reduce(out=rmax[:], in_=lgs[:],
                                        axis=mybir.AxisListType.X,
                                        op=mybir.AluOpType.max)
                xm = smpool.tile([P, N_CLS], f32, tag="xm")
                nc.vector.tensor_scalar(out=xm[:], in0=lgs[:],
                                        scalar1=rmax[:, 0:1], scalar2=None,
                                        op0=mybir.AluOpType.subtract)
                ex = smpool.tile([P, N_CLS], f32, tag="ex")
                nc.scalar.activation(ex[:], xm[:],
                                     mybir.ActivationFunctionType.Exp)
                sume = smpool.tile([P, 1], f32, tag="sume")
                nc.vector.tensor_reduce(out=sume[:], in_=ex[:],
                                        axis=mybir.AxisListType.X,
                                        op=mybir.AluOpType.add)
                lse = smpool.tile([P, 1], f32, tag="lse")
                nc.scalar.activation(lse[:], sume[:],
                                     mybir.ActivationFunctionType.Ln)
                res = smpool.tile([P, N_CLS], f32, tag="res")
                nc.vector.tensor_scalar(out=res[:], in0=xm[:],
                                        scalar1=lse[:, 0:1], scalar2=None,
                                        op0=mybir.AluOpType.subtract)
                nc.sync.dma_start(out[half * P:(half + 1) * P, :], res[:])

    nc.compile()

    # Post-schedule queue realignment: Tile assigns SWDGE completion sems
    # round-robin over 8 DMASW lanes in SCHEDULED order. A lane must always
    # serve the same hw queue, else cross-queue completion reordering can
    # satisfy a cumulative wait threshold before an earlier same-lane DMA
    # has landed. Pin each lane's queue to lane%NQ -- except lanes that host
    # an InstDMACopy (indirect scatter), which always executes on queue 0.
    pool_dmas = []
    for blk in nc.m.functions[0].blocks:
        for inst in blk.instructions:
            if inst.engine != mybir.EngineType.Pool:
                continue
            tname = type(inst).__name__
            if tname not in ("InstDMAGatherAnt", "InstDMACopy"):
                continue
            lane = None
            si = inst.sync_info
            if si:
                for u in si.on_update:
                    if u.ant_name and u.ant_name.startswith("DMASW"):
                        lane = int(u.ant_name[5:].split("_")[0])
            if lane is not None:
                pool_dmas.append((inst, tname, lane))
    lane_q = {lane: lane % NQ for _, _, lane in pool_dmas}
    for _, tname, lane in pool_dmas:
        if tname == "InstDMACopy":
            lane_q[lane] = 0
    for inst, tname, lane in pool_dmas:
        if tname == "InstDMAGatherAnt":
            inst.queue_num = lane_q[lane]
        else:
            inst.queue = "qPoolDynamic"
    return nc


# ------------------------------------------------------------------ runner
def _make_runner(nc, n_cores):
    import jax
    import concourse.mybir as mybir
    from jax.experimental.shard_map import shard_map
    from jax.sharding import Mesh, NamedSharding, PartitionSpec
    from concourse.bass2jax import (_bass_exec_p, install_neuronx_cc_hook,
                                    partition_id_tensor)

    install_neuronx_cc_hook()
    partition_name = nc.partition_id_tensor.name if nc.partition_id_tensor else None
    in_names, out_names, out_avals = [], [], []
    for alloc in nc.m.functions[0].allocations:
        if not isinstance(alloc, mybir.MemoryLocationSet):
            continue
        name = alloc.memorylocations[0].name
        if alloc.kind == "ExternalInput":
            if name != partition_name:
                in_names.append(name)
        elif alloc.kind == "ExternalOutput":
            out_names.append(name)
            out_avals.append(jax.core.ShapedArray(
                tuple(alloc.tensor_shape), mybir.dt.np(alloc.dtype)))
    n_params = len(in_names)
    all_in = list(in_names) + list(out_names)
    if partition_name is not None:
        all_in.append(partition_name)

    def _body(*args):
        operands = list(args)
        if partition_name is not None:
            operands.append(partition_id_tensor())
        return tuple(_bass_exec_p.bind(
            *operands, out_avals=tuple(out_avals), in_names=tuple(all_in),
            out_names=tuple(out_names), lowering_input_output_aliases=(),
            sim_require_finite=False, sim_require_nnan=False, nc=nc))

    devices = jax.devices()[:n_cores]
    mesh = Mesh(np.asarray(devices), ("core",))
    nin = n_params + len(out_names)
    sharded = jax.jit(shard_map(
        _body, mesh=mesh, in_specs=(PartitionSpec("core"),) * nin,
        out_specs=(PartitionSpec("core"),) * len(out_names), check_rep=False),
        keep_unused=True)
    sharding = NamedSharding(mesh, PartitionSpec("core"))

    def stage(in_maps):
        import jax as _jax
        concat_in = [np.concatenate([np.asarray(in_maps[c][nm])
                                     for c in range(n_cores)], axis=0)
                     for nm in in_names]
        concat_zero = [np.zeros((n_cores * a.shape[0], *a.shape[1:]), a.dtype)
                       for a in out_avals]
        return [_jax.device_put(x, sharding) for x in concat_in + concat_zero]

    def call(staged):
        import jax as _jax
        outs = sharded(*staged)
        _jax.block_until_ready(outs)
        return outs

    def fetch(outs):
        return [{nm: np.asarray(outs[i]).reshape(n_cores, *out_avals[i].shape)[c]
                 for i, nm in enumerate(out_names)} for c in range(n_cores)]

    return stage, call, fetch


_CACHE = {}


def _get_compiled(st):
    key = (st["S_core"], st["C_CH"], tuple(st["cpg"].tolist()))
    if key not in _CACHE:
        nc = _build(st)
        _CACHE[key] = (nc, _make_runner(nc, NCORES))
    return _CACHE[key]


def _in_maps(st, per_core, x, inputs):
    node_slot = st["node_slot"]
    S_core = st["S_core"]
    hom_W = np.asarray(inputs["hom_W"], np.float32)
    het_W = np.asarray(inputs["het_W"], np.float32)
    hom_b = np.asarray(inputs["hom_b"], np.float32)
    het_b = np.asarray(inputs["het_b"], np.float32)
    w2 = np.ascontiguousarray(np.concatenate([hom_W[:, 2], het_W[:, 2]], axis=2))
    w1 = np.ascontiguousarray(np.concatenate([hom_W[:, 1], het_W[:, 1]], axis=2))
    w0 = np.ascontiguousarray(np.concatenate([hom_W[:, 0], het_W[:, 0]], axis=2))
    bb = np.ascontiguousarray(np.concatenate([hom_b, het_b], axis=1)[:, None, :])
    rcount = np.zeros((NGP, 1), np.float32)
    rcount[:N_GRAPHS, 0] = 1.0 / np.maximum(st["counts"], 1.0)
    consts = _host_consts(st)

    x = np.asarray(x, np.float32)
    maps = []
    for c in range(NCORES):
        xo = np.zeros((S_core, F_IN), np.float32)
        m = (node_slot >= c * S_core) & (node_slot < (c + 1) * S_core)
        xo[node_slot[m] - c * S_core] = x[m]
        pc = per_core[c]
        maps.append({
            "x_own": np.ascontiguousarray(xo.T), "idx16": pc["idx16"], "dlA": pc["dl"],
            "mqA": pc["mq"], "rdegA": pc["rdeg"], "batchloc": pc["batchloc"],
            "maskrow": pc["maskrow"], "gidx": pc["gidx"], "rcount_g": rcount,
            "iotaDG": consts["iotaDG"], "iotaWG": consts["iotaWG"],
            "identA": consts["identA"],
            "w2cat": w2, "w1cat": w1, "w0cat": w0, "bcat": bb,
            "lin1w": np.asarray(inputs["lin1_W"], np.float32),
            "lin1b": np.asarray(inputs["lin1_b"], np.float32)[:, None],
            "lin2w": np.asarray(inputs["lin2_W"], np.float32),
            "lin2b": np.asarray(inputs["lin2_b"], np.float32)[:, None],
            "lin3w": np.asarray(inputs["lin3_W"], np.float32),
            "lin3b": np.asarray(inputs["lin3_b"], np.float32)[:, None]})
    return maps


def kernel(**inputs):
    x = np.asarray(inputs["x"])
    edge_index = np.asarray(inputs["edge_index"])
    batch = np.asarray(inputs["batch"])
    st, per_core = _prep(edge_index, batch, inputs["hom_mask"], inputs["het_mask"])
    nc, (stage, call, fetch) = _get_compiled(st)
    maps = _in_maps(st, per_core, x, inputs)
    staged = stage(maps)
    outs = call(staged)
    return fetch(outs)[0]["out"].astype(np.float32)
